# revision 2
# baseline (speedup 1.0000x reference)
"""DSAM (dual spatial/channel attention) Bass kernel for Trainium2, 8 cores.

Sharding: core c handles batch b=c//4, query-row quarter qi=c%4
(1024 of the 4096 spatial positions). Spatial attention is fused
flash-style (scores -> exp -> weighted sum of V, normalization folded in
via an appended ones-row of V). The channel branch (full-image 3x3 conv +
64x64 gram) is computed redundantly per core.

Host sends f16 data directly (no on-device dtype conversion):
 - xd    [65,4096]  dense image + ones row (1x1 convs k/v)
 - xsd   [65,1024]  dense our-quarter + ones row (q)
 - sf    [128,LF]   padded image slab: lower 64 partitions = x,
                    upper 64 = x shifted +66 (one padded row)
 - sf2   [128,LF2]  same, shifted +132 / +133
 - sq/sq2           our 18 padded rows, same doubled layout
The doubled slabs let 3x3 convs run as 5 PE passes (4 tap-pairs with
K=128 + 1 single tap) instead of 9; conv bias is folded into the DVE
relu-eviction (tensor_scalar add+max).

Program order weaves the channel-branch conv blocks, transposes, gram,
and channel softmax into the 32 attention rounds so PE keeps streaming
while ACT runs the exp stream (the long pole).

A post-pass enforces the 1-sync-wait-per-engine-instruction encoding
limit via per-engine FIFO elision / same-sem merging.

Hardcoded shapes: B=2, C=64, H=W=64, Cq=8.
"""

import numpy as np

EPS = 1e-5
B, C, H, W = 2, 64, 64, 64
HW = H * W
Cq = C // 8
NQ = 1024

LF = 4360
LF2 = 4224
LQ = 1192
LQ2 = 1058

# d_w column layout (f16 [128, WLEN])
O_WPS = 0            # wpk_s [128, 5, 64]
O_WPC = 320          # wpk_c [128, 5, 64]
O_WQ = 640           # [65, 8]
O_WK = 648           # [65, 8]
O_WV = 656           # [65, 65]
O_WO = 721           # [64, 64]
O_ID = 785           # [64, 64]
O_ONE = 849          # [1, 64] ones
WLEN = 913

_CACHE = {}


def _build():
    import concourse.bass as bass
    import concourse.tile as tile
    from concourse import mybir
    from contextlib import ExitStack

    fp = mybir.dt.float32
    f16 = mybir.dt.float16
    AX = mybir.AxisListType.X
    ALU = mybir.AluOpType
    ACTF = mybir.ActivationFunctionType

    nc = bass.Bass()
    d_w = nc.dram_tensor("wpk", [128, WLEN], f16, kind="ExternalInput")
    d_c = nc.dram_tensor("cst", [64, 4], fp, kind="ExternalInput")
    d_xsd = nc.dram_tensor("xsd", [65, NQ], f16, kind="ExternalInput")
    d_xd = nc.dram_tensor("xd", [65, HW], f16, kind="ExternalInput")
    d_sq = nc.dram_tensor("sq", [128, LQ], f16, kind="ExternalInput")
    d_sq2 = nc.dram_tensor("sq2", [128, LQ2], f16, kind="ExternalInput")
    d_sf = nc.dram_tensor("sf", [128, LF], f16, kind="ExternalInput")
    d_sf2 = nc.dram_tensor("sf2", [128, LF2], f16, kind="ExternalInput")
    out_d = nc.dram_tensor("out", [64, NQ], fp, kind="ExternalOutput")

    with tile.TileContext(nc) as tc, ExitStack() as ctx:
        big = ctx.enter_context(tc.tile_pool(name="big", bufs=1))
        work = ctx.enter_context(tc.tile_pool(name="work", bufs=3))
        small = ctx.enter_context(tc.tile_pool(name="small", bufs=8))
        ps_s = ctx.enter_context(tc.tile_pool(name="ps_s", bufs=2, space="PSUM"))
        ps_u = ctx.enter_context(tc.tile_pool(name="ps_u", bufs=2, space="PSUM"))
        ps_m = ctx.enter_context(tc.tile_pool(name="ps_m", bufs=2, space="PSUM"))

        # ---------------- input DMAs (order = arrival order) ----------------
        w_sb = big.tile([128, WLEN], f16)
        nc.gpsimd.dma_start(w_sb, d_w[:, :])
        c_sb = big.tile([64, 4], fp)
        nc.gpsimd.dma_start(c_sb, d_c[:, :])
        xsd = big.tile([65, NQ], f16)
        nc.gpsimd.dma_start(xsd, d_xsd[:, :])
        xd = big.tile([65, HW], f16)
        nc.gpsimd.dma_start(xd, d_xd[:, :])
        sq = big.tile([128, LQ], f16)
        nc.gpsimd.dma_start(sq, d_sq[:, :])
        sq2 = big.tile([128, LQ2], f16)
        nc.gpsimd.dma_start(sq2, d_sq2[:, :])
        sf = big.tile([128, LF], f16)
        nc.gpsimd.dma_start(sf, d_sf[:, :])
        sf2 = big.tile([128, LF2], f16)
        nc.gpsimd.dma_start(sf2, d_sf2[:, :])

        wpk_s = w_sb[:, O_WPS:O_WPS + 320].rearrange("c (g o) -> c g o", g=5)
        wpk_c = w_sb[:, O_WPC:O_WPC + 320].rearrange("c (g o) -> c g o", g=5)
        wq8 = w_sb[:65, O_WQ:O_WQ + Cq]
        wk8 = w_sb[:65, O_WK:O_WK + Cq]
        wv = w_sb[:65, O_WV:O_WV + 65]
        wo16 = w_sb[:64, O_WO:O_WO + 64]
        ident = w_sb[:64, O_ID:O_ID + 64]
        ones16 = w_sb[0:1, O_ONE:O_ONE + 64]
        sbias = c_sb[:, 0:1]
        cbias = c_sb[:, 1:2]
        ob = c_sb[:, 2:3]
        cg = c_sb[:, 3:4]

        # early DVE touch: seeds FIFO wait-coverage of the d_c DMA so later
        # DVE instructions' d_c waits are elided by the post-pass
        cscr = big.tile([64, 4], fp)
        nc.vector.tensor_copy(cscr, c_sb)

        # persistent SBUF tensors
        k_sb = big.tile([Cq, HW], f16)
        q_sb = big.tile([Cq, NQ], f16)
        vT = big.tile([128, 32, 65], f16)
        cxf = big.tile([64, HW], f16)      # full-image channel fmap (dense)
        fT = big.tile([128, 32, 64], f16)  # fmap transposed chunks
        sxq = big.tile([64, NQ], f16)      # spatial-conv output, our rows
        cxq = big.tile([64, NQ], f16)      # channel-conv output, our rows
        fuse = big.tile([64, NQ], f16)
        out_sb = big.tile([64, NQ], fp)

        # ---------------- emitters ------------------------------------------
        def emit_q(blk):
            ps = ps_m.tile([128, 512], fp, tag="m", name="m")
            nc.tensor.matmul(ps[:Cq, :], wq8,
                             xsd[:, blk * 512:(blk + 1) * 512],
                             start=True, stop=True)
            nc.scalar.copy(q_sb[:, blk * 512:(blk + 1) * 512], ps[:Cq, :])

        def emit_k(blk):
            ps = ps_m.tile([128, 512], fp, tag="m", name="m")
            nc.tensor.matmul(ps[:Cq, :], wk8,
                             xd[:, blk * 512:(blk + 1) * 512],
                             start=True, stop=True)
            nc.scalar.copy(k_sb[:, blk * 512:(blk + 1) * 512], ps[:Cq, :])

        def emit_vt(grp):
            n_t = min(7, 32 - grp * 7)
            ps = ps_m.tile([128, 512], fp, tag="m", name="m")
            for t in range(n_t):
                jo = grp * 7 + t
                nc.tensor.matmul(ps[:, t * 65:(t + 1) * 65],
                                 xd[:, jo * 128:(jo + 1) * 128], wv,
                                 start=True, stop=True)
            nc.scalar.copy(vT[:, grp * 7:grp * 7 + n_t, :], ps[:, :n_t * 65])

        # paired-tap 3x3 conv block: 5 matmuls. sA = [x | x<<66] slab,
        # sB = [x<<132 | x<<133] slab, base = padded-flat offset of the
        # first output position, fsz = rows*66. The sB pair is emitted last
        # so a late-arriving sB DMA doesn't stall the earlier taps.
        def conv_mms(ps, wpk, sA, sB, base, fsz):
            nc.tensor.matmul(ps[:64, :fsz], wpk[:, 0, :],
                             sA[:, base: base + fsz], start=True, stop=False)
            nc.tensor.matmul(ps[:64, :fsz], wpk[:, 1, :],
                             sA[:, base + 1: base + 1 + fsz],
                             start=False, stop=False)
            nc.tensor.matmul(ps[:64, :fsz], wpk[:, 2, :],
                             sA[:, base + 2: base + 2 + fsz],
                             start=False, stop=False)
            nc.tensor.matmul(ps[:64, :fsz], wpk[:64, 4, :],
                             sA[:64, base + 134: base + 134 + fsz],
                             start=False, stop=False)
            nc.tensor.matmul(ps[:64, :fsz], wpk[:, 3, :],
                             sB[:, base: base + fsz], start=False, stop=True)

        F_ROWS = [7] * 9 + [1]

        def emit_fblock(b):
            rows = F_ROWS[b]
            done = 7 * b
            fsz = rows * 66
            ps = ps_m.tile([128, 512], fp, tag="m", name="m")
            conv_mms(ps, wpk_c, sf, sf2, done * 66, fsz)
            pv = ps[:64, :fsz].rearrange("c (r w) -> c r w", w=66)[:, :, 1:65]
            nc.vector.tensor_scalar(
                cxf[:, done * 64:(done + rows) * 64].rearrange(
                    "c (r w) -> c r w", w=64),
                pv, cbias, 0.0, ALU.add, ALU.max)

        def emit_qconv(bi, wpk, dst, bias):
            rows = (7, 7, 2)[bi]
            p0 = (0, 462, 924)[bi]
            fsz = rows * 66
            ps = ps_m.tile([128, 512], fp, tag="m", name="m")
            conv_mms(ps, wpk, sq, sq2, p0, fsz)
            pv = ps[:64, :fsz].rearrange("c (r w) -> c r w", w=66)[:, :, 1:65]
            nc.vector.tensor_scalar(
                dst[:, p0 // 66 * 64:(p0 // 66 + rows) * 64].rearrange(
                    "c (r w) -> c r w", w=64),
                pv, bias, 0.0, ALU.add, ALU.max)

        def emit_ftrans(grp):
            ps = ps_m.tile([128, 512], f16, tag="m", name="m")
            for t in range(8):
                jo = grp * 8 + t
                nc.tensor.transpose(ps[:, t * 64:(t + 1) * 64],
                                    cxf[:, jo * 128:(jo + 1) * 128], ident)
            nc.vector.tensor_copy(fT[:, grp * 8:(grp + 1) * 8, :], ps)

        gram_ps = {}

        def emit_gram(grp):
            if 'ps' not in gram_ps:
                gram_ps['ps'] = ps_u.tile([65, 512], fp, tag="U", name="U")
            psA = gram_ps['ps'][:64, :64]
            for t in range(8):
                jo = grp * 8 + t
                nc.tensor.matmul(psA, fT[:, jo, :], fT[:, jo, :],
                                 start=(jo == 0), stop=(jo == 31))

        chan = {}

        def emit_chansoft():
            psA = gram_ps['ps'][:64, :64]
            Ac = small.tile([64, 64], fp, name="ac")
            nc.vector.tensor_copy(Ac, psA)
            mn = small.tile([64, 1], fp, name="mn")
            nc.vector.tensor_reduce(mn, Ac, AX, ALU.min)
            Ec = small.tile([64, 64], f16, name="ec")
            # exp(mn - Ac): softmax(max-Ac) == softmax(-Ac), stabilized by min
            nc.scalar.activation(Ec, Ac, ACTF.Exp, bias=mn, scale=-1.0)
            sm = small.tile([64, 1], fp, name="sm")
            nc.vector.reduce_sum(sm, Ec, AX)
            rc = small.tile([64, 1], fp, name="rc")
            nc.vector.reciprocal(rc, sm)
            # Ec := Ec * (1/sum) * c_gamma
            nc.vector.tensor_scalar(Ec, Ec, rc, cg, ALU.mult, ALU.mult)
            psT = ps_m.tile([128, 512], f16, tag="m", name="m")
            nc.tensor.transpose(psT[:64, :64], Ec, ident)
            ScT = small.tile([64, 64], f16, name="sct")
            nc.vector.tensor_copy(ScT, psT[:64, :64])
            chan['ScT'] = ScT

        psS = {}

        def emit_S(r):
            ib, rnd = divmod(r, 16)
            ps = ps_s.tile([128, 1024], fp, tag="S", name="S")
            for hh in range(2):
                jo = rnd * 2 + hh
                nc.tensor.matmul(ps[:, hh * 512:(hh + 1) * 512],
                                 k_sb[:, jo * 128:(jo + 1) * 128],
                                 q_sb[:, ib * 512:(ib + 1) * 512],
                                 start=True, stop=True)
            psS[r] = ps

        # ---------------- startup -------------------------------------------
        emit_q(0)
        emit_q(1)
        emit_k(0)
        emit_k(1)
        emit_vt(0)
        emit_S(0)
        emit_S(1)

        # weave schedule: round -> list of thunks
        weave = {}

        def at(r, fn, *a):
            weave.setdefault(r, []).append((fn, a))

        for i, b in enumerate((2, 3, 4, 5, 6, 7)):
            at((0, 1, 2, 4, 6, 8)[i], emit_k, b)
        at(0, emit_vt, 1)
        at(2, emit_vt, 2)
        at(4, emit_vt, 3)
        at(6, emit_vt, 4)
        at(4, emit_qconv, 0, wpk_s, sxq, sbias)
        at(5, emit_qconv, 1, wpk_s, sxq, sbias)
        at(6, emit_qconv, 2, wpk_s, sxq, sbias)
        for b in range(10):
            at(6 + b, emit_fblock, b)
        at(9, emit_ftrans, 0)
        at(11, emit_ftrans, 1)
        at(14, emit_ftrans, 2)
        at(16, emit_ftrans, 3)
        at(17, emit_gram, 0)
        at(18, emit_gram, 1)
        at(19, emit_gram, 2)
        at(20, emit_gram, 3)
        at(21, emit_chansoft)
        at(22, emit_qconv, 0, wpk_c, cxq, cbias)
        at(23, emit_qconv, 1, wpk_c, cxq, cbias)
        at(24, emit_qconv, 2, wpk_c, cxq, cbias)

        # ---------------- attention rounds ----------------------------------
        psU = {}
        for r in range(32):
            ib, rnd = divmod(r, 16)
            Et = work.tile([128, 1024], f16, tag="E", name="E")
            nc.scalar.activation(Et, psS.pop(r), ACTF.Exp)
            if rnd == 0:
                psU[ib] = ps_u.tile([65, 512], fp, tag="U", name="U")
            for hh in range(2):
                jo = rnd * 2 + hh
                nc.tensor.matmul(psU[ib], vT[:, jo, :],
                                 Et[:, hh * 512:(hh + 1) * 512],
                                 start=(jo == 0), stop=(jo == 31))
            for fn, a in weave.get(r, ()):
                fn(*a)
            if r + 2 < 32:
                emit_S(r + 2)
            if rnd == 15:
                # spatial tail: fuse = psU * (1/denom) + sxq
                rcp = small.tile([1, 512], f16, name="rec")
                with nc.allow_low_precision(reason="1/denom via f16 matmul"):
                    nc.vector.reciprocal(rcp, psU[ib][64:65, :])
                psB = ps_m.tile([128, 512], fp, tag="m", name="m")
                nc.tensor.matmul(psB[:64, :], ones16, rcp,
                                 start=True, stop=True)
                rec64 = small.tile([64, 512], f16, name="rec64")
                nc.vector.tensor_copy(rec64, psB[:64, :])
                fb = fuse[:, ib * 512:(ib + 1) * 512]
                nc.vector.tensor_tensor(fb, psU[ib][:64, :], rec64, ALU.mult)
                nc.vector.tensor_add(fb, fb, sxq[:, ib * 512:(ib + 1) * 512])

        # ---------------- channel apply + output conv ----------------------
        for ib in range(2):
            psC = ps_m.tile([128, 512], fp, tag="m", name="m")
            nc.tensor.matmul(psC[:64, :], chan['ScT'],
                             cxq[:, ib * 512:(ib + 1) * 512],
                             start=True, stop=True)
            fb = fuse[:, ib * 512:(ib + 1) * 512]
            nc.vector.tensor_add(fb, fb, psC[:64, :])
            nc.vector.tensor_add(fb, fb, cxq[:, ib * 512:(ib + 1) * 512])
            psO = ps_m.tile([128, 512], fp, tag="m", name="m")
            nc.tensor.matmul(psO[:64, :], wo16, fb, start=True, stop=True)
            nc.vector.tensor_scalar_add(
                out_sb[:, ib * 512:(ib + 1) * 512], psO[:64, :], ob)
        nc.gpsimd.dma_start(out_d[:, :], out_sb)

    _postpass(nc)
    return nc


def _postpass(nc):
    """Enforce <=1 sync wait per engine instruction.

    Safe transforms only:
     - merge same-sem waits to the max target value;
     - drop a wait (sem, v) if an EARLIER same-engine instruction already
       guaranteed sem >= v (FIFO queues, monotone sems);
     - drop a wait (sem, v) if another wait (sem2, v2) on the same
       instruction transitively covers it: the engine owning sem2 had
       already guaranteed sem >= v by the time its update count hit v2;
     - move a surplus input-DMA wait to the immediately preceding
       same-engine instruction when it has a free slot (input DMAs depend
       on nothing, so no cycle can form); same for any wait when the host
       is an Ldweights (nothing can depend on an Ldweights).
    """
    import bass_rust
    eng_names = ('PE', 'Activation', 'DVE', 'Pool')
    skip_types = ('InstEventSemaphore', 'InstDrain')
    sem_eng = {'PE_': 'PE', 'Activation_': 'Activation', 'DVE_': 'DVE',
               'Pool_': 'Pool'}

    def eng_of_sem(nm):
        for p, e in sem_eng.items():
            if nm.startswith(p):
                return e
        return None

    seen = {e: {} for e in eng_names}    # sem -> max value guaranteed
    snap = {e: {} for e in eng_names}    # update count -> seen snapshot
    cnt = {e: 0 for e in eng_names}      # cumulative own-sem update count
    prev = {e: None for e in eng_names}
    last_dma_sem = None
    for blk in nc.m.functions[0].blocks:
        for ins in blk.instructions:
            tname = type(ins).__name__
            eng = str(getattr(ins, 'engine', '')).replace('EngineType.', '')
            si = ins.sync_info
            if si is not None:
                for u in si.on_update:
                    if u.ant_name.startswith('DMA'):
                        last_dma_sem = u.ant_name
            if eng not in eng_names or tname in skip_types:
                continue
            if si is None:
                prev[eng] = ins
                continue
            sn = seen[eng]
            merged = {}
            for w in si.on_wait:
                nm = w.ant_name
                v = w.wait_value or 0
                if nm not in merged or v > (merged[nm].wait_value or 0):
                    merged[nm] = w
            # all merged targets are guaranteed at/after this instruction
            implied = dict(merged)
            # drop waits an earlier same-engine instruction guaranteed
            kept = [w for w in merged.values()
                    if sn.get(w.ant_name, -1) < (w.wait_value or 0)]
            # own-engine waits >=4 updates back are covered by FIFO spacing
            # (pipeline drain ~190ns << 4 instructions of engine occupancy)
            kept = [w for w in kept
                    if not (eng_of_sem(w.ant_name) == eng
                            and (w.wait_value or 0) <= cnt[eng] - 4)]
            # transitive coverage between remaining waits
            if len(kept) > 1:
                def covered(w, others):
                    for o in others:
                        e2 = eng_of_sem(o.ant_name)
                        if e2 is None:
                            continue
                        s2 = snap[e2].get(o.wait_value or 0)
                        if s2 and s2.get(w.ant_name, -1) >= (w.wait_value or 0):
                            return True
                    return False
                kept = [w for w in kept
                        if not covered(w, [o for o in kept if o is not w])]
            for nm, w in implied.items():
                sn[nm] = max(sn.get(nm, -1), w.wait_value or 0)
            if len(kept) > 1:
                p = prev[eng]
                assert p is not None, (ins.name, eng, tname)
                p_w = list(p.sync_info.on_wait) if p.sync_info else []
                p_names = {w.ant_name for w in p_w}
                ok_lw = type(p).__name__ == 'InstLdweights'
                plan = None
                for keep_w in kept:
                    move = [w for w in kept if w is not keep_w]
                    names = p_names | {w.ant_name for w in move}
                    if len(names) > 1 or (len(p_w) + len(move) > 1
                                          and len(names) > 1):
                        # merged-by-name result must fit in one wait slot
                        if not (len(names) <= 1):
                            # allowed only via LW / pure-DMA move with free p
                            if not ((ok_lw or all(
                                    w.ant_name.startswith('DMASW')
                                    for w in move)) and not p_w
                                    and len(move) <= 1):
                                continue
                    plan = (keep_w, move)
                    break
                assert plan is not None, \
                    (ins.name, eng, tname,
                     [(w.ant_name, w.wait_value) for w in kept],
                     p.name, type(p).__name__,
                     [(w.ant_name, w.wait_value) for w in p_w])
                keep_w, move = plan
                newpw = {}
                for w in p_w + move:
                    nm = w.ant_name
                    if nm not in newpw or (w.wait_value or 0) > \
                            (newpw[nm].wait_value or 0):
                        newpw[nm] = w
                psi = p.sync_info
                if psi is None:
                    psi = bass_rust.SyncInfo(on_wait=[], on_update=[])
                psi.on_wait = list(newpw.values())
                p.sync_info = psi
                kept = [keep_w]
            si.on_wait = kept
            ins.sync_info = si
            prev[eng] = ins
            for u in si.on_update:
                if u.ant_name == f'{eng}_44' or (
                        eng_of_sem(u.ant_name) == eng):
                    cnt[eng] += (u.update_value or 1)
                    snap[eng][cnt[eng]] = dict(seen[eng])
    # tail drains: the final out DMA transitively covers every engine
    for blk in nc.m.functions[0].blocks:
        for ins in blk.instructions:
            si = ins.sync_info
            if si is None or type(ins).__name__ != 'InstDrain':
                continue
            if len(si.on_wait) > 1 and last_dma_sem is not None:
                keep = [w for w in si.on_wait if w.ant_name == last_dma_sem]
                if keep:
                    si.on_wait = keep
                    ins.sync_info = si


def _prep_host(inputs):
    x = np.asarray(inputs['x'], np.float32)

    def fold(Wc, bc, g, b_, m, v):
        sc = np.asarray(g) / np.sqrt(np.asarray(v) + EPS)
        return (np.asarray(Wc) * sc[:, None, None, None],
                (np.asarray(bc) - np.asarray(m)) * sc + np.asarray(b_))

    sWf, sbf = fold(inputs['sW'], inputs['sb'], inputs['s_g'], inputs['s_b'],
                    inputs['s_m'], inputs['s_v'])
    cWf, cbf = fold(inputs['cW'], inputs['cb'], inputs['c_g'], inputs['c_b'],
                    inputs['c_m'], inputs['c_v'])

    def pack_pairs(Wf):
        # tap (dy,dx) weight matrix [cin, cout] = Wf[:, :, dy, dx].T
        T = [Wf[:, :, dy, dx].T for dy in range(3) for dx in range(3)]
        g = np.zeros((128, 5, 64), np.float32)
        g[:64, 0], g[64:, 0] = T[0], T[3]
        g[:64, 1], g[64:, 1] = T[1], T[4]
        g[:64, 2], g[64:, 2] = T[2], T[5]
        g[:64, 3], g[64:, 3] = T[6], T[7]
        g[:64, 4] = T[8]
        return g.reshape(128, 320)

    w16 = np.zeros((128, WLEN), np.float32)
    w16[:, O_WPS:O_WPS + 320] = pack_pairs(sWf)
    w16[:, O_WPC:O_WPC + 320] = pack_pairs(cWf)
    w16[:64, O_WQ:O_WQ + Cq] = np.asarray(inputs['qW'])[:, :, 0, 0].T
    w16[64, O_WQ:O_WQ + Cq] = np.asarray(inputs['qb'])
    w16[:64, O_WK:O_WK + Cq] = np.asarray(inputs['kW'])[:, :, 0, 0].T
    w16[64, O_WK:O_WK + Cq] = np.asarray(inputs['kb'])
    sg = float(np.asarray(inputs['s_gamma'])[0])
    w16[:64, O_WV:O_WV + 64] = np.asarray(inputs['vW'])[:, :, 0, 0].T * sg
    w16[64, O_WV:O_WV + 64] = np.asarray(inputs['vb']) * sg
    w16[64, O_WV + 64] = 1.0
    w16[:64, O_WO:O_WO + 64] = np.asarray(inputs['oW'])[:, :, 0, 0].T
    w16[:64, O_ID:O_ID + 64] = np.eye(64, dtype=np.float32)
    w16[0, O_ONE:O_ONE + 64] = 1.0
    w16 = w16.astype(np.float16)

    c32 = np.zeros((64, 4), np.float32)
    c32[:, 0] = sbf
    c32[:, 1] = cbf
    c32[:, 2] = np.asarray(inputs['ob'])
    c32[:, 3] = float(np.asarray(inputs['c_gamma'])[0])

    in_maps = []
    for core in range(8):
        b, qi = core // 4, core % 4
        xb = x[b].astype(np.float16)
        xd = np.ones((65, HW), np.float16)
        xd[:64] = xb.reshape(64, HW)
        xsd = np.ones((65, NQ), np.float16)
        xsd[:64] = xb[:, qi * 16:(qi + 1) * 16, :].reshape(64, NQ)
        # padded flat with leading sentinel: z[1 + r*66 + w] = xpad[r, w]
        xp = np.zeros((64, 66, 66), np.float16)
        xp[:, 1:65, 1:65] = xb
        z = np.zeros((64, 4426 + 66), np.float16)
        z[:, 1:1 + 4356] = xp.reshape(64, 4356)
        sfm = np.zeros((128, LF), np.float16)
        sfm[:64] = z[:, 0:LF]
        sfm[64:] = z[:, 66:66 + LF]
        sf2m = np.zeros((128, LF2), np.float16)
        sf2m[:64] = z[:, 132:132 + LF2]
        sf2m[64:] = z[:, 133:133 + LF2]
        # our-quarter slab: padded rows [16qi, 16qi+18)
        zq = np.zeros((64, LQ + 200), np.float16)
        zq[:, 1:1 + 18 * 66] = xp[:, qi * 16:qi * 16 + 18, :].reshape(64, -1)
        sqm = np.zeros((128, LQ), np.float16)
        sqm[:64] = zq[:, 0:LQ]
        sqm[64:] = zq[:, 66:66 + LQ]
        sq2m = np.zeros((128, LQ2), np.float16)
        sq2m[:64] = zq[:, 132:132 + LQ2]
        sq2m[64:] = zq[:, 133:133 + LQ2]
        in_maps.append({
            'wpk': w16, 'cst': c32,
            'xsd': np.ascontiguousarray(xsd),
            'xd': np.ascontiguousarray(xd),
            'sq': sqm, 'sq2': sq2m, 'sf': sfm, 'sf2': sf2m,
        })
    return in_maps


def kernel(**inputs):
    from concourse.bass_utils import run_bass_kernel_spmd
    if 'nc' not in _CACHE:
        _CACHE['nc'] = _build()
    nc = _CACHE['nc']
    in_maps = _prep_host(inputs)
    res = run_bass_kernel_spmd(nc, in_maps, core_ids=list(range(8)))
    out = np.zeros((B, C, H, W), np.float32)
    for c in range(8):
        b, qi = c // 4, c % 4
        out[b, :, qi * 16:(qi + 1) * 16, :] = \
            res.results[c]['out'].reshape(64, 16, 64)
    return out


# revision 3
# speedup vs baseline: 1.1711x; 1.1711x over previous
"""DSAM (dual spatial/channel attention) Bass kernel for Trainium2, 8 cores.

Sharding: core c handles batch b=c//4, query-row quarter qi=c%4
(1024 of the 4096 spatial positions). Spatial attention is fused
flash-style (scores -> exp -> weighted sum of V, normalization folded in
via an appended ones-row of V). The channel branch (full-image 3x3 conv +
64x64 gram) is computed redundantly per core.

Key optimizations over the 95.5us baseline:
 - host sends f16 data directly (no on-device dtype conversion);
 - doubled slabs [x | x<<66] and [x<<132 | x<<133] let every 3x3 conv run
   as 5 PE passes (4 tap-pairs with K=128 + 1 single) instead of 9, with
   conv bias folded into the DVE relu-eviction;
 - the whole spatial attention runs in fp8e4 with DoubleRow matmuls
   (2 contraction tiles per pass): scores contract k/q over [4,2,*]
   layouts, A@V contracts vT/exp(Et) over [128,2,*] - halving PE time;
   fp8 is numerically free here because the softmax self-normalizes;
 - input DMAs are HWDGE on the idle SP engine, ordered by first use;
   outputs stream per-half;
 - channel-branch work (conv blocks, transposes, gram, softmax) and the
   ca+cxq+sxq pre-sums are woven between attention rounds so PE/DVE run
   under the ACT exp stream (the long pole);
 - a post-pass enforces the 1-sync-wait-per-engine-instruction encoding
   limit (FIFO elision, transitive coverage, same-sem merge).

Hardcoded shapes: B=2, C=64, H=W=64, Cq=8.
"""

import numpy as np

EPS = 1e-5
B, C, H, W = 2, 64, 64, 64
HW = H * W
Cq = C // 8
NQ = 1024

LF = 4360
LF2 = 4224
LQ = 1192
LQ2 = 1058

# d_w column layout (f16 [128, WLEN])
O_WPS = 0            # wpk_s [128, 5, 64]
O_WPC = 320          # wpk_c [128, 5, 64]
O_WQ = 640           # [65, 8]
O_WK = 648           # [65, 8]
O_WV = 656           # [65, 65]
O_WO = 721           # [64, 64]
O_ID = 785           # [64, 64]
O_ONE = 849          # [1, 64] ones
WLEN = 913

_CACHE = {}


def _build():
    import concourse.bass as bass
    import concourse.tile as tile
    from concourse import mybir
    from contextlib import ExitStack

    fp = mybir.dt.float32
    f16 = mybir.dt.float16
    f8 = mybir.dt.float8e4
    AX = mybir.AxisListType.X
    ALU = mybir.AluOpType
    ACTF = mybir.ActivationFunctionType
    DR = mybir.MatmulPerfMode.DoubleRow

    nc = bass.Bass()
    d_w = nc.dram_tensor("wpk", [128, WLEN], f16, kind="ExternalInput")
    d_c = nc.dram_tensor("cst", [64, 4], fp, kind="ExternalInput")
    d_xd = nc.dram_tensor("xd", [65, HW], f16, kind="ExternalInput")
    d_xsd = nc.dram_tensor("xsd", [65, NQ], f16, kind="ExternalInput")
    d_sq = nc.dram_tensor("sq", [128, LQ], f16, kind="ExternalInput")
    d_sq2 = nc.dram_tensor("sq2", [128, LQ2], f16, kind="ExternalInput")
    d_sf = nc.dram_tensor("sf", [128, LF], f16, kind="ExternalInput")
    d_sf2 = nc.dram_tensor("sf2", [128, LF2], f16, kind="ExternalInput")
    out_d = nc.dram_tensor("out", [64, NQ], fp, kind="ExternalOutput")

    with tile.TileContext(nc) as tc, ExitStack() as ctx:
        big = ctx.enter_context(tc.tile_pool(name="big", bufs=1))
        work = ctx.enter_context(tc.tile_pool(name="work", bufs=3))
        small = ctx.enter_context(tc.tile_pool(name="small", bufs=10))
        ps_s = ctx.enter_context(tc.tile_pool(name="ps_s", bufs=2, space="PSUM"))
        ps_u = ctx.enter_context(tc.tile_pool(name="ps_u", bufs=2, space="PSUM"))
        ps_m = ctx.enter_context(tc.tile_pool(name="ps_m", bufs=2, space="PSUM"))

        # ------------- input DMAs: SP-engine HWDGE, arrival order -----------
        w_sb = big.tile([128, WLEN], f16)
        nc.sync.dma_start(w_sb, d_w[:, :])
        c_sb = big.tile([64, 4], fp)
        nc.sync.dma_start(c_sb, d_c[:, :])
        xd = big.tile([65, HW], f16)
        nc.sync.dma_start(xd, d_xd[:, :])
        xsd = big.tile([65, NQ], f16)
        nc.sync.dma_start(xsd, d_xsd[:, :])
        sq = big.tile([128, LQ], f16)
        nc.sync.dma_start(sq, d_sq[:, :])
        sq2 = big.tile([128, LQ2], f16)
        nc.sync.dma_start(sq2, d_sq2[:, :])
        sf = big.tile([128, LF], f16)
        nc.sync.dma_start(sf, d_sf[:, :])
        sf2 = big.tile([128, LF2], f16)
        nc.sync.dma_start(sf2, d_sf2[:, :])

        wpk_s = w_sb[:, O_WPS:O_WPS + 320].rearrange("c (g o) -> c g o", g=5)
        wpk_c = w_sb[:, O_WPC:O_WPC + 320].rearrange("c (g o) -> c g o", g=5)
        wq8 = w_sb[:65, O_WQ:O_WQ + Cq]
        wk8 = w_sb[:65, O_WK:O_WK + Cq]
        wv = w_sb[:65, O_WV:O_WV + 65]
        wo16 = w_sb[:64, O_WO:O_WO + 64]
        ident = w_sb[:64, O_ID:O_ID + 64]
        ones16 = w_sb[0:1, O_ONE:O_ONE + 64]
        sbias = c_sb[:, 0:1]
        cbias = c_sb[:, 1:2]
        ob = c_sb[:, 2:3]
        cg = c_sb[:, 3:4]

        # early DVE touch: seeds FIFO wait-coverage of the d_c DMA so later
        # DVE instructions' d_c waits are elided by the post-pass
        cscr = big.tile([64, 4], fp)
        nc.vector.tensor_copy(cscr, c_sb)

        # persistent SBUF tensors
        k_dr = big.tile([4, 2, HW], f8)    # k, DoubleRow layout: ch = t*4+p
        q_dr = big.tile([4, 2, NQ], f8)
        vT = big.tile([128, 32, 128], f8)
        # zero vT's pad columns once (Pool is idle early); evictions only
        # write cols 0:65, the pad keeps DoubleRow M=128 exact
        nc.gpsimd.memset(vT[:, :, 65:128], 0.0)
        cxf = big.tile([64, HW], f16)      # full-image channel fmap (dense)
        fT = big.tile([128, 32, 64], f16)  # fmap transposed chunks
        sxq = big.tile([64, NQ], f16)      # spatial-conv output, our rows
        cxq = big.tile([64, NQ], f16)      # channel-conv output, our rows
        pre = big.tile([64, NQ], f16)      # ca + cxq + sxq, precomputed
        fuse = big.tile([64, NQ], f16)
        out_sb = big.tile([64, NQ], fp)

        # ---------------- emitters ------------------------------------------
        def emit_q(t, half):
            ps = ps_m.tile([128, 512], fp, tag="m", name="m")
            nc.tensor.matmul(ps[:4, :], wq8[:, t * 4:(t + 1) * 4],
                             xsd[:, half * 512:(half + 1) * 512],
                             start=True, stop=True)
            nc.vector.tensor_copy(q_dr[:, t, half * 512:(half + 1) * 512],
                                  ps[:4, :])

        def emit_k(blk):
            for t in range(2):
                ps = ps_m.tile([128, 512], fp, tag="m", name="m")
                nc.tensor.matmul(ps[:4, :], wk8[:, t * 4:(t + 1) * 4],
                                 xd[:, blk * 512:(blk + 1) * 512],
                                 start=True, stop=True)
                nc.vector.tensor_copy(
                    k_dr[:, t, blk * 512:(blk + 1) * 512], ps[:4, :])

        def emit_vt(grp):
            n_t = min(7, 32 - grp * 7)
            ps = ps_m.tile([128, 512], fp, tag="m", name="m")
            for t in range(n_t):
                jo = grp * 7 + t
                nc.tensor.matmul(ps[:, t * 65:(t + 1) * 65],
                                 xd[:, jo * 128:(jo + 1) * 128], wv,
                                 start=True, stop=True)
            nc.scalar.copy(vT[:, grp * 7:grp * 7 + n_t, 0:65],
                           ps[:, :n_t * 65])

        # paired-tap 3x3 conv block: 5 matmuls. sA = [x | x<<66] slab,
        # sB = [x<<132 | x<<133] slab. The sB pair is emitted last so a
        # late-arriving sB DMA doesn't stall the earlier taps.
        def conv_mms(ps, wpk, sA, sB, base, fsz):
            nc.tensor.matmul(ps[:64, :fsz], wpk[:, 0, :],
                             sA[:, base: base + fsz], start=True, stop=False)
            nc.tensor.matmul(ps[:64, :fsz], wpk[:, 1, :],
                             sA[:, base + 1: base + 1 + fsz],
                             start=False, stop=False)
            nc.tensor.matmul(ps[:64, :fsz], wpk[:, 2, :],
                             sA[:, base + 2: base + 2 + fsz],
                             start=False, stop=False)
            nc.tensor.matmul(ps[:64, :fsz], wpk[:64, 4, :],
                             sA[:64, base + 134: base + 134 + fsz],
                             start=False, stop=False)
            nc.tensor.matmul(ps[:64, :fsz], wpk[:, 3, :],
                             sB[:, base: base + fsz], start=False, stop=True)

        F_ROWS = [7] * 9 + [1]

        def emit_fblock(b):
            rows = F_ROWS[b]
            done = 7 * b
            fsz = rows * 66
            ps = ps_m.tile([128, 512], fp, tag="m", name="m")
            conv_mms(ps, wpk_c, sf, sf2, done * 66, fsz)
            pv = ps[:64, :fsz].rearrange("c (r w) -> c r w", w=66)[:, :, 1:65]
            nc.vector.tensor_scalar(
                cxf[:, done * 64:(done + rows) * 64].rearrange(
                    "c (r w) -> c r w", w=64),
                pv, cbias, 0.0, ALU.add, ALU.max)

        def emit_qconv(bi, wpk, dst, bias):
            rows = (7, 7, 2)[bi]
            p0 = (0, 462, 924)[bi]
            fsz = rows * 66
            ps = ps_m.tile([128, 512], fp, tag="m", name="m")
            conv_mms(ps, wpk, sq, sq2, p0, fsz)
            pv = ps[:64, :fsz].rearrange("c (r w) -> c r w", w=66)[:, :, 1:65]
            nc.vector.tensor_scalar(
                dst[:, p0 // 66 * 64:(p0 // 66 + rows) * 64].rearrange(
                    "c (r w) -> c r w", w=64),
                pv, bias, 0.0, ALU.add, ALU.max)

        def emit_ftrans(grp):
            ps = ps_m.tile([128, 512], f16, tag="m", name="m")
            for t in range(8):
                jo = grp * 8 + t
                nc.tensor.transpose(ps[:, t * 64:(t + 1) * 64],
                                    cxf[:, jo * 128:(jo + 1) * 128], ident)
            nc.vector.tensor_copy(fT[:, grp * 8:(grp + 1) * 8, :], ps)

        gram_ps = {}

        def emit_gram(grp):
            if 'ps' not in gram_ps:
                gram_ps['ps'] = ps_u.tile([128, 512], fp, tag="U", name="U")
            psA = gram_ps['ps'][:64, :64]
            for t in range(8):
                jo = grp * 8 + t
                nc.tensor.matmul(psA, fT[:, jo, :], fT[:, jo, :],
                                 start=(jo == 0), stop=(jo == 31))

        chan = {}

        def emit_chansoft():
            psA = gram_ps['ps'][:64, :64]
            Ac = small.tile([64, 64], fp, name="ac")
            nc.vector.tensor_copy(Ac, psA)
            mn = small.tile([64, 1], fp, name="mn")
            nc.vector.tensor_reduce(mn, Ac, AX, ALU.min)
            Ec = small.tile([64, 64], f16, name="ec")
            # exp(mn - Ac): softmax(max-Ac) == softmax(-Ac), stabilized by min
            nc.scalar.activation(Ec, Ac, ACTF.Exp, bias=mn, scale=-1.0)
            sm = small.tile([64, 1], fp, name="sm")
            nc.vector.reduce_sum(sm, Ec, AX)
            rc = small.tile([64, 1], fp, name="rc")
            nc.vector.reciprocal(rc, sm)
            # Ec := Ec * (1/sum) * c_gamma
            nc.vector.tensor_scalar(Ec, Ec, rc, cg, ALU.mult, ALU.mult)
            psT = ps_m.tile([128, 512], f16, tag="m", name="m")
            nc.tensor.transpose(psT[:64, :64], Ec, ident)
            ScT = small.tile([64, 64], f16, name="sct")
            nc.vector.tensor_copy(ScT, psT[:64, :64])
            chan['ScT'] = ScT

        def emit_pre(ib):
            # pre = gamma_c*softmax(..) @ cxq + cxq + sxq; the +cxq rides the
            # same PSUM via an identity-matmul accumulate
            psC = ps_m.tile([128, 512], fp, tag="m", name="m")
            cx = cxq[:, ib * 512:(ib + 1) * 512]
            nc.tensor.matmul(psC[:64, :], chan['ScT'], cx,
                             start=True, stop=False)
            nc.tensor.matmul(psC[:64, :], ident, cx, start=False, stop=True)
            nc.vector.tensor_tensor(pre[:, ib * 512:(ib + 1) * 512],
                                    psC[:64, :],
                                    sxq[:, ib * 512:(ib + 1) * 512], ALU.add)

        def emit_out(ib):
            fb = fuse[:, ib * 512:(ib + 1) * 512]
            nc.vector.tensor_add(fb, fb, pre[:, ib * 512:(ib + 1) * 512])
            psO = ps_m.tile([128, 512], fp, tag="m", name="m")
            nc.tensor.matmul(psO[:64, :], wo16, fb, start=True, stop=True)
            nc.vector.tensor_scalar_add(
                out_sb[:, ib * 512:(ib + 1) * 512], psO[:64, :], ob)
            nc.sync.dma_start(out_d[:, ib * 512:(ib + 1) * 512],
                              out_sb[:, ib * 512:(ib + 1) * 512])

        psS = {}

        def emit_S(r):
            ib, rnd = divmod(r, 16)
            ps = ps_s.tile([128, 1024], fp, tag="S", name="S")
            for hh in range(2):
                jo = rnd * 2 + hh
                nc.tensor.matmul(ps[:, hh * 512:(hh + 1) * 512],
                                 k_dr[:, :, jo * 128:(jo + 1) * 128],
                                 q_dr[:, :, ib * 512:(ib + 1) * 512],
                                 start=True, stop=True, perf_mode=DR)
            psS[r] = ps

        # ---------------- startup -------------------------------------------
        emit_k(0)
        emit_q(0, 0)
        emit_q(1, 0)
        emit_q(0, 1)
        emit_q(1, 1)
        emit_k(1)
        emit_vt(0)
        emit_S(0)
        emit_S(1)

        weave = {}

        def at(r, fn, *a):
            weave.setdefault(r, []).append((fn, a))

        for i, b in enumerate((2, 3, 4, 5, 6, 7)):
            at(i, emit_k, b)
        at(0, emit_vt, 1)
        at(1, emit_vt, 2)
        at(2, emit_vt, 3)
        at(3, emit_vt, 4)
        at(2, emit_qconv, 0, wpk_s, sxq, sbias)
        at(3, emit_qconv, 1, wpk_s, sxq, sbias)
        at(4, emit_qconv, 2, wpk_s, sxq, sbias)
        at(5, emit_qconv, 0, wpk_c, cxq, cbias)
        at(6, emit_qconv, 1, wpk_c, cxq, cbias)
        at(7, emit_qconv, 2, wpk_c, cxq, cbias)
        for b in range(10):
            at(8 + b, emit_fblock, b)
        at(11, emit_ftrans, 0)
        at(13, emit_ftrans, 1)
        at(16, emit_ftrans, 2)
        at(18, emit_ftrans, 3)
        at(17, emit_gram, 0)
        at(18, emit_gram, 1)
        at(19, emit_gram, 2)
        at(20, emit_gram, 3)
        at(21, emit_chansoft)
        at(23, emit_pre, 0)
        at(23, emit_pre, 1)
        at(24, emit_out, 0)

        # ---------------- attention rounds ----------------------------------
        psU = {}
        for r in range(32):
            ib, rnd = divmod(r, 16)
            Et = work.tile([128, 1024], f8, tag="E", name="E")
            nc.scalar.activation(Et, psS.pop(r), ACTF.Exp)
            if rnd == 0:
                psU[ib] = ps_u.tile([128, 512], fp, tag="U", name="U")
            nc.tensor.matmul(psU[ib], vT[:, 2 * rnd:2 * rnd + 2, :],
                             Et.rearrange("c (t n) -> c t n", t=2),
                             start=(rnd == 0), stop=(rnd == 15), perf_mode=DR)
            for fn, a in weave.get(r, ()):
                fn(*a)
            if r + 2 < 32:
                emit_S(r + 2)
            if rnd == 15:
                # fuse = psU * (1/denom); (+pre) added in emit_out
                rcp = small.tile([1, 512], f16, name="rec")
                with nc.allow_low_precision(reason="1/denom via f16 matmul"):
                    nc.vector.reciprocal(rcp, psU[ib][64:65, :])
                psB = ps_m.tile([128, 512], fp, tag="m", name="m")
                nc.tensor.matmul(psB[:64, :], ones16, rcp,
                                 start=True, stop=True)
                rec64 = small.tile([64, 512], f16, name="rec64")
                nc.vector.tensor_copy(rec64, psB[:64, :])
                fb = fuse[:, ib * 512:(ib + 1) * 512]
                nc.vector.tensor_tensor(fb, psU[ib][:64, :], rec64, ALU.mult)

        emit_out(1)

    _postpass(nc)
    return nc


def _postpass(nc):
    """Enforce <=1 sync wait per engine instruction.

    Safe transforms only:
     - merge same-sem waits to the max target value;
     - drop a wait (sem, v) if an EARLIER same-engine instruction already
       guaranteed sem >= v (FIFO queues, monotone sems);
     - drop an own-engine wait >=4 updates back (pipeline drain ~190ns is
       far less than 4 instructions of engine occupancy);
     - drop a wait (sem, v) if another wait (sem2, v2) on the same
       instruction transitively covers it: the engine owning sem2 had
       already guaranteed sem >= v by the time its update count hit v2;
     - move a surplus wait to the immediately preceding same-engine
       instruction when the result still fits one wait slot per
       instruction: either merged into a same-sem wait there, or the host
       is an Ldweights / the moved wait is an input-DMA sem (input DMAs
       depend on nothing, Ldweights have no dependents - no cycles).
       Own-engine-sem waits never move (could self-deadlock).
    """
    import bass_rust
    eng_names = ('PE', 'Activation', 'DVE', 'Pool', 'SP')
    skip_types = ('InstEventSemaphore', 'InstDrain')
    sem_eng = {'PE_': 'PE', 'Activation_': 'Activation', 'DVE_': 'DVE',
               'Pool_': 'Pool'}

    def eng_of_sem(nm):
        for p, e in sem_eng.items():
            if nm.startswith(p):
                return e
        return None

    seen = {e: {} for e in eng_names}    # sem -> max value guaranteed
    snap = {e: {} for e in eng_names}    # update count -> seen snapshot
    cnt = {e: 0 for e in eng_names}      # cumulative own-sem update count
    prev = {e: None for e in eng_names}
    last_dma_sem = None
    for blk in nc.m.functions[0].blocks:
        for ins in blk.instructions:
            tname = type(ins).__name__
            eng = str(getattr(ins, 'engine', '')).replace('EngineType.', '')
            si = ins.sync_info
            if si is not None:
                for u in si.on_update:
                    if u.ant_name.startswith('DMA'):
                        last_dma_sem = u.ant_name
            if eng not in eng_names or tname in skip_types:
                continue
            if eng == 'SP' and tname != 'InstDMACopy':
                continue
            if si is None:
                prev[eng] = ins
                continue
            sn = seen[eng]
            merged = {}
            for w in si.on_wait:
                nm = w.ant_name
                v = w.wait_value or 0
                if nm not in merged or v > (merged[nm].wait_value or 0):
                    merged[nm] = w
            implied = dict(merged)
            kept = [w for w in merged.values()
                    if sn.get(w.ant_name, -1) < (w.wait_value or 0)]
            kept = [w for w in kept
                    if not (eng_of_sem(w.ant_name) == eng
                            and (w.wait_value or 0) <= cnt[eng] - 4)]
            if len(kept) > 1:
                def covered(w, others):
                    for o in others:
                        e2 = eng_of_sem(o.ant_name)
                        if e2 is None:
                            continue
                        s2 = snap[e2].get(o.wait_value or 0)
                        if s2 and s2.get(w.ant_name, -1) >= (w.wait_value or 0):
                            return True
                    return False
                kept = [w for w in kept
                        if not covered(w, [o for o in kept if o is not w])]
            for nm, w in implied.items():
                sn[nm] = max(sn.get(nm, -1), w.wait_value or 0)
            if len(kept) > 1:
                p = prev[eng]
                assert p is not None, (ins.name, eng, tname)
                p_w = list(p.sync_info.on_wait) if p.sync_info else []
                p_names = {w.ant_name for w in p_w}
                ok_lw = type(p).__name__ == 'InstLdweights'
                plan = None
                for keep_w in kept:
                    move = [w for w in kept if w is not keep_w]
                    if any(eng_of_sem(w.ant_name) == eng for w in move):
                        continue  # never move own-engine waits
                    names = p_names | {w.ant_name for w in move}
                    if len(names) <= 1:
                        plan = (keep_w, move)
                        break
                    if not p_w and len(move) == 1 and (
                            ok_lw or move[0].ant_name.startswith('DMA')):
                        plan = (keep_w, move)
                        break
                assert plan is not None, \
                    (ins.name, eng, tname,
                     [(w.ant_name, w.wait_value) for w in kept],
                     p.name, type(p).__name__,
                     [(w.ant_name, w.wait_value) for w in p_w])
                keep_w, move = plan
                newpw = {}
                for w in p_w + move:
                    nm = w.ant_name
                    if nm not in newpw or (w.wait_value or 0) > \
                            (newpw[nm].wait_value or 0):
                        newpw[nm] = w
                psi = p.sync_info
                if psi is None:
                    psi = bass_rust.SyncInfo(on_wait=[], on_update=[])
                psi.on_wait = list(newpw.values())
                p.sync_info = psi
                kept = [keep_w]
            si.on_wait = kept
            ins.sync_info = si
            prev[eng] = ins
            for u in si.on_update:
                if eng_of_sem(u.ant_name) == eng:
                    cnt[eng] += (u.update_value or 1)
                    snap[eng][cnt[eng]] = dict(seen[eng])
    # tail drains: the final out DMA transitively covers every engine
    for blk in nc.m.functions[0].blocks:
        for ins in blk.instructions:
            si = ins.sync_info
            if si is None or type(ins).__name__ != 'InstDrain':
                continue
            if len(si.on_wait) > 1 and last_dma_sem is not None:
                keep = [w for w in si.on_wait if w.ant_name == last_dma_sem]
                if keep:
                    si.on_wait = keep
                    ins.sync_info = si


def _prep_host(inputs):
    x = np.asarray(inputs['x'], np.float32)

    def fold(Wc, bc, g, b_, m, v):
        sc = np.asarray(g) / np.sqrt(np.asarray(v) + EPS)
        return (np.asarray(Wc) * sc[:, None, None, None],
                (np.asarray(bc) - np.asarray(m)) * sc + np.asarray(b_))

    sWf, sbf = fold(inputs['sW'], inputs['sb'], inputs['s_g'], inputs['s_b'],
                    inputs['s_m'], inputs['s_v'])
    cWf, cbf = fold(inputs['cW'], inputs['cb'], inputs['c_g'], inputs['c_b'],
                    inputs['c_m'], inputs['c_v'])

    def pack_pairs(Wf):
        # tap (dy,dx) weight matrix [cin, cout] = Wf[:, :, dy, dx].T
        T = [Wf[:, :, dy, dx].T for dy in range(3) for dx in range(3)]
        g = np.zeros((128, 5, 64), np.float32)
        g[:64, 0], g[64:, 0] = T[0], T[3]
        g[:64, 1], g[64:, 1] = T[1], T[4]
        g[:64, 2], g[64:, 2] = T[2], T[5]
        g[:64, 3], g[64:, 3] = T[6], T[7]
        g[:64, 4] = T[8]
        return g.reshape(128, 320)

    w16 = np.zeros((128, WLEN), np.float32)
    w16[:, O_WPS:O_WPS + 320] = pack_pairs(sWf)
    w16[:, O_WPC:O_WPC + 320] = pack_pairs(cWf)
    w16[:64, O_WQ:O_WQ + Cq] = np.asarray(inputs['qW'])[:, :, 0, 0].T
    w16[64, O_WQ:O_WQ + Cq] = np.asarray(inputs['qb'])
    w16[:64, O_WK:O_WK + Cq] = np.asarray(inputs['kW'])[:, :, 0, 0].T
    w16[64, O_WK:O_WK + Cq] = np.asarray(inputs['kb'])
    sg = float(np.asarray(inputs['s_gamma'])[0])
    w16[:64, O_WV:O_WV + 64] = np.asarray(inputs['vW'])[:, :, 0, 0].T * sg
    w16[64, O_WV:O_WV + 64] = np.asarray(inputs['vb']) * sg
    w16[64, O_WV + 64] = 1.0
    w16[:64, O_WO:O_WO + 64] = np.asarray(inputs['oW'])[:, :, 0, 0].T
    w16[:64, O_ID:O_ID + 64] = np.eye(64, dtype=np.float32)
    w16[0, O_ONE:O_ONE + 64] = 1.0
    w16 = w16.astype(np.float16)

    c32 = np.zeros((64, 4), np.float32)
    c32[:, 0] = sbf
    c32[:, 1] = cbf
    c32[:, 2] = np.asarray(inputs['ob'])
    c32[:, 3] = float(np.asarray(inputs['c_gamma'])[0])

    in_maps = []
    for core in range(8):
        b, qi = core // 4, core % 4
        xb = x[b].astype(np.float16)
        xdm = np.ones((65, HW), np.float16)
        xdm[:64] = xb.reshape(64, HW)
        xsdm = np.ones((65, NQ), np.float16)
        xsdm[:64] = xb[:, qi * 16:(qi + 1) * 16, :].reshape(64, NQ)
        # padded flat with leading sentinel: z[1 + r*66 + w] = xpad[r, w]
        xp = np.zeros((64, 66, 66), np.float16)
        xp[:, 1:65, 1:65] = xb
        z = np.zeros((64, 4426 + 66), np.float16)
        z[:, 1:1 + 4356] = xp.reshape(64, 4356)
        sfm = np.zeros((128, LF), np.float16)
        sfm[:64] = z[:, 0:LF]
        sfm[64:] = z[:, 66:66 + LF]
        sf2m = np.zeros((128, LF2), np.float16)
        sf2m[:64] = z[:, 132:132 + LF2]
        sf2m[64:] = z[:, 133:133 + LF2]
        # our-quarter slab: padded rows [16qi, 16qi+18)
        zq = np.zeros((64, LQ + 200), np.float16)
        zq[:, 1:1 + 18 * 66] = xp[:, qi * 16:qi * 16 + 18, :].reshape(64, -1)
        sqm = np.zeros((128, LQ), np.float16)
        sqm[:64] = zq[:, 0:LQ]
        sqm[64:] = zq[:, 66:66 + LQ]
        sq2m = np.zeros((128, LQ2), np.float16)
        sq2m[:64] = zq[:, 132:132 + LQ2]
        sq2m[64:] = zq[:, 133:133 + LQ2]
        in_maps.append({
            'wpk': w16, 'cst': c32,
            'xd': np.ascontiguousarray(xdm),
            'xsd': np.ascontiguousarray(xsdm),
            'sq': sqm, 'sq2': sq2m, 'sf': sfm, 'sf2': sf2m,
        })
    return in_maps


def kernel(**inputs):
    from concourse.bass_utils import run_bass_kernel_spmd
    if 'nc' not in _CACHE:
        _CACHE['nc'] = _build()
    nc = _CACHE['nc']
    in_maps = _prep_host(inputs)
    res = run_bass_kernel_spmd(nc, in_maps, core_ids=list(range(8)))
    out = np.zeros((B, C, H, W), np.float32)
    for c in range(8):
        b, qi = c // 4, c % 4
        out[b, :, qi * 16:(qi + 1) * 16, :] = \
            res.results[c]['out'].reshape(64, 16, 64)
    return out


# revision 6
# speedup vs baseline: 1.3766x; 1.1755x over previous
"""DSAM (dual spatial/channel attention) Bass kernel for Trainium2, 8 cores.

Sharding: core c handles batch b=c//4, query-row quarter qi=c%4
(1024 of the 4096 spatial positions). Spatial attention is fused
flash-style (scores -> exp -> weighted sum of V, normalization folded in
via an appended ones-row of V). The channel branch (full-image 3x3 conv +
64x64 gram) is computed redundantly per core.

Key optimizations over the 95.5us baseline:
 - host sends f16 data directly (no on-device dtype conversion);
 - doubled slabs [x | x<<66] and [x<<132 | x<<133] let every 3x3 conv run
   as 5 PE passes (4 tap-pairs with K=128 + 1 single) instead of 9, with
   conv bias folded into the DVE relu-eviction;
 - the whole spatial attention runs in fp8e4 with DoubleRow matmuls
   (2 contraction tiles per pass): scores contract k/q over [4,2,*]
   layouts, A@V contracts vT/exp(Et) over [128,2,*] - halving PE time;
   fp8 is numerically free here because the softmax self-normalizes;
 - input DMAs are HWDGE on the idle SP engine, ordered by first use;
   outputs stream per-half;
 - channel-branch work (conv blocks, transposes, gram, softmax) and the
   ca+cxq+sxq pre-sums are woven between attention rounds so PE/DVE run
   under the ACT exp stream (the long pole);
 - a post-pass enforces the 1-sync-wait-per-engine-instruction encoding
   limit (FIFO elision, transitive coverage, same-sem merge).

Hardcoded shapes: B=2, C=64, H=W=64, Cq=8.
"""

import numpy as np

EPS = 1e-5
B, C, H, W = 2, 64, 64, 64
HW = H * W
Cq = C // 8
NQ = 1024

LF = 4360
LF2 = 4224
LQ = 1192
LQ2 = 1058

# d_w0 column layout (f16 [65, W0LEN]) - tiny, arrives first
O_KQ = 0             # wkq_t [65, 2, 33]: (Wk @ Wq^T) halves, transposed
O_WV = 66            # [65, 65]
W0LEN = 131
# d_w column layout (f16 [128, WLEN])
O_WPS = 0            # wpk_s [128, 5, 64]
O_WPC = 320          # wpk_c [128, 5, 64]
O_WO = 640           # [64, 64]
O_ID = 704           # [64, 64]
O_ONE = 768          # [1, 64] ones
WLEN = 832

_CACHE = {}


def _build():
    import concourse.bass as bass
    import concourse.tile as tile
    from concourse import mybir
    from contextlib import ExitStack

    fp = mybir.dt.float32
    f16 = mybir.dt.float16
    f8 = mybir.dt.float8e4
    AX = mybir.AxisListType.X
    ALU = mybir.AluOpType
    ACTF = mybir.ActivationFunctionType
    DR = mybir.MatmulPerfMode.DoubleRow

    nc = bass.Bass()
    d_w0 = nc.dram_tensor("wpk0", [65, W0LEN], f16, kind="ExternalInput")
    d_w = nc.dram_tensor("wpk", [128, WLEN], f16, kind="ExternalInput")
    d_c = nc.dram_tensor("cst", [64, 4], fp, kind="ExternalInput")
    d_xd = nc.dram_tensor("xd", [65, HW], f16, kind="ExternalInput")
    d_xdr = nc.dram_tensor("xdr", [33, 2 * HW], mybir.dt.float8e4,
                           kind="ExternalInput")
    d_xsd = nc.dram_tensor("xsd", [65, NQ], f16, kind="ExternalInput")
    d_sq = nc.dram_tensor("sq", [128, LQ], f16, kind="ExternalInput")
    d_sq2 = nc.dram_tensor("sq2", [128, LQ2], f16, kind="ExternalInput")
    d_sf = nc.dram_tensor("sf", [128, LF], f16, kind="ExternalInput")
    d_sf2 = nc.dram_tensor("sf2", [128, LF2], f16, kind="ExternalInput")
    out_d = nc.dram_tensor("out", [64, NQ], fp, kind="ExternalOutput")

    with tile.TileContext(nc) as tc, ExitStack() as ctx:
        big = ctx.enter_context(tc.tile_pool(name="big", bufs=1))
        work = ctx.enter_context(tc.tile_pool(name="work", bufs=5))
        small = ctx.enter_context(tc.tile_pool(name="small", bufs=12))
        ps_s = ctx.enter_context(tc.tile_pool(name="ps_s", bufs=2, space="PSUM"))
        ps_u = ctx.enter_context(tc.tile_pool(name="ps_u", bufs=2, space="PSUM"))
        ps_m = ctx.enter_context(tc.tile_pool(name="ps_m", bufs=2, space="PSUM"))

        # ------------- input DMAs, desc-gen spread across idle queues -------
        # (HWDGE desc-gen costs ~650ns of SEQ time per DMA and serializes per
        # engine; the transfers themselves serialize on the DMA engines in
        # arrival order, so queue assignment = arrival priority)
        xsd = big.tile([65, NQ], f16)
        nc.scalar.dma_start(xsd, d_xsd[:, :])
        w0_sb = big.tile([65, W0LEN], f16)
        nc.sync.dma_start(w0_sb, d_w0[:, :])
        xd_dr = big.tile([33, 2, HW], f8)
        nc.sync.dma_start(xd_dr,
                          d_xdr[:, :].rearrange("c (t n) -> c t n", t=2))
        xd = big.tile([65, HW], f16)
        nc.scalar.dma_start(xd, d_xd[:, :])
        c_sb = big.tile([64, 4], fp)
        nc.sync.dma_start(c_sb, d_c[:, :])
        w_sb = big.tile([128, WLEN], f16)
        nc.sync.dma_start(w_sb, d_w[:, :])
        wup = big.tile([128, 512], f16)
        nc.gpsimd.memset(wup, 0.0)
        vT = big.tile([128, 32, 128], f8)
        nc.gpsimd.memset(vT[:, :, 65:128], 0.0)
        sq = big.tile([128, LQ], f16)
        nc.gpsimd.dma_start(sq, d_sq[:, :])
        sq2 = big.tile([128, LQ2], f16)
        nc.gpsimd.dma_start(sq2, d_sq2[:, :])
        sf = big.tile([128, LF], f16)
        nc.gpsimd.dma_start(sf, d_sf[:, :])
        sf2 = big.tile([128, LF2], f16)
        nc.gpsimd.dma_start(sf2, d_sf2[:, :])

        wpk_s = w_sb[:, O_WPS:O_WPS + 320].rearrange("c (g o) -> c g o", g=5)
        wpk_c = w_sb[:, O_WPC:O_WPC + 320].rearrange("c (g o) -> c g o", g=5)
        wv = w0_sb[:65, O_WV:O_WV + 65]
        wo16 = w_sb[:64, O_WO:O_WO + 64]
        ident = w_sb[:64, O_ID:O_ID + 64]
        ones16 = w_sb[0:1, O_ONE:O_ONE + 64]
        wkq = w0_sb[:65, O_KQ:O_KQ + 66].rearrange("c (t m) -> c t m", t=2)
        sbias = c_sb[:, 0:1]
        cbias = c_sb[:, 1:2]
        ob = c_sb[:, 2:3]
        cg = c_sb[:, 3:4]

        # early DVE touch: seeds FIFO wait-coverage of the d_c DMA so later
        # DVE instructions' d_c waits are elided by the post-pass
        cscr = big.tile([64, 4], fp)
        nc.vector.tensor_copy(cscr, c_sb)

        # persistent SBUF tensors
        qk_dr = big.tile([33, 2, NQ], f8)  # (Wk Wq^T xsd), DoubleRow layout
        cxf = big.tile([64, HW], f16)      # full-image channel fmap (dense)
        fT = big.tile([128, 32, 64], f16)  # fmap transposed chunks
        sxq = big.tile([64, NQ], f16)      # spatial-conv output, our rows
        cxq = big.tile([64, NQ], f16)      # channel-conv output, our rows
        pre = big.tile([64, NQ], f16)      # ca + cxq + sxq, precomputed
        fuse = big.tile([64, NQ], f16)
        out_sb = big.tile([64, NQ], fp)

        # ---------------- emitters ------------------------------------------
        def emit_qk(t, half):
            ps = ps_m.tile([128, 512], fp, tag="m", name="m")
            nc.tensor.matmul(ps[:33, :], wkq[:, t, :33],
                             xsd[:, half * 512:(half + 1) * 512],
                             start=True, stop=True)
            nc.vector.tensor_copy(qk_dr[:, t, half * 512:(half + 1) * 512],
                                  ps[:33, :])

        def emit_vt(grp):
            n_t = min(7, 32 - grp * 7)
            ps = ps_m.tile([128, 512], fp, tag="m", name="m")
            for t in range(n_t):
                jo = grp * 7 + t
                nc.tensor.matmul(ps[:, t * 65:(t + 1) * 65],
                                 xd[:, jo * 128:(jo + 1) * 128], wv,
                                 start=True, stop=True)
            nc.vector.tensor_copy(vT[:, grp * 7:grp * 7 + n_t, 0:65],
                                  ps[:, :n_t * 65])

        # paired-tap 3x3 conv block: 5 matmuls. sA = [x | x<<66] slab,
        # sB = [x<<132 | x<<133] slab. The sB pair is emitted last so a
        # late-arriving sB DMA doesn't stall the earlier taps.
        def conv_mms(ps, wpk, sA, sB, base, fsz):
            nc.tensor.matmul(ps[:64, :fsz], wpk[:, 0, :],
                             sA[:, base: base + fsz], start=True, stop=False)
            nc.tensor.matmul(ps[:64, :fsz], wpk[:, 1, :],
                             sA[:, base + 1: base + 1 + fsz],
                             start=False, stop=False)
            nc.tensor.matmul(ps[:64, :fsz], wpk[:, 2, :],
                             sA[:, base + 2: base + 2 + fsz],
                             start=False, stop=False)
            nc.tensor.matmul(ps[:64, :fsz], wpk[:64, 4, :],
                             sA[:64, base + 134: base + 134 + fsz],
                             start=False, stop=False)
            nc.tensor.matmul(ps[:64, :fsz], wpk[:, 3, :],
                             sB[:, base: base + fsz], start=False, stop=True)

        F_ROWS = [7] * 9 + [1]

        def emit_fblock(b):
            rows = F_ROWS[b]
            done = 7 * b
            fsz = rows * 66
            ps = ps_m.tile([128, 512], fp, tag="m", name="m")
            conv_mms(ps, wpk_c, sf, sf2, done * 66, fsz)
            pv = ps[:64, :fsz].rearrange("c (r w) -> c r w", w=66)[:, :, 1:65]
            nc.vector.tensor_scalar(
                cxf[:, done * 64:(done + rows) * 64].rearrange(
                    "c (r w) -> c r w", w=64),
                pv, cbias, 0.0, ALU.add, ALU.max)

        def emit_qconv(bi, wpk, dst, bias):
            rows = (7, 7, 2)[bi]
            p0 = (0, 462, 924)[bi]
            fsz = rows * 66
            ps = ps_m.tile([128, 512], fp, tag="m", name="m")
            conv_mms(ps, wpk, sq, sq2, p0, fsz)
            pv = ps[:64, :fsz].rearrange("c (r w) -> c r w", w=66)[:, :, 1:65]
            nc.vector.tensor_scalar(
                dst[:, p0 // 66 * 64:(p0 // 66 + rows) * 64].rearrange(
                    "c (r w) -> c r w", w=64),
                pv, bias, 0.0, ALU.add, ALU.max)

        def emit_ftrans(grp):
            ps = ps_m.tile([128, 512], f16, tag="m", name="m")
            for t in range(8):
                jo = grp * 8 + t
                nc.tensor.transpose(ps[:, t * 64:(t + 1) * 64],
                                    cxf[:, jo * 128:(jo + 1) * 128], ident)
            nc.vector.tensor_copy(fT[:, grp * 8:(grp + 1) * 8, :], ps)

        gram_ps = {}

        def emit_gram(grp):
            if 'ps' not in gram_ps:
                gram_ps['ps'] = ps_u.tile([128, 512], fp, tag="U", name="U")
            psA = gram_ps['ps'][:64, :64]
            for t in range(8):
                jo = grp * 8 + t
                nc.tensor.matmul(psA, fT[:, jo, :], fT[:, jo, :],
                                 start=(jo == 0), stop=(jo == 31))

        chan = {}

        def emit_chansoft():
            psA = gram_ps['ps'][:64, :64]
            Ac = small.tile([64, 64], fp, name="ac")
            nc.vector.tensor_copy(Ac, psA)
            mn = small.tile([64, 1], fp, name="mn")
            nc.vector.tensor_reduce(mn, Ac, AX, ALU.min)
            Ec = small.tile([64, 64], f16, name="ec")
            # exp(mn - Ac): softmax(max-Ac) == softmax(-Ac), stabilized by min
            nc.scalar.activation(Ec, Ac, ACTF.Exp, bias=mn, scale=-1.0)
            sm = small.tile([64, 1], fp, name="sm")
            nc.vector.reduce_sum(sm, Ec, AX)
            rc = small.tile([64, 1], fp, name="rc")
            nc.vector.reciprocal(rc, sm)
            # Ec := Ec * (1/sum) * c_gamma
            nc.vector.tensor_scalar(Ec, Ec, rc, cg, ALU.mult, ALU.mult)
            psT = ps_m.tile([128, 512], f16, tag="m", name="m")
            nc.tensor.transpose(psT[:64, :64], Ec, ident)
            ScT = small.tile([64, 64], f16, name="sct")
            nc.vector.tensor_copy(ScT, psT[:64, :64])
            chan['ScT'] = ScT

        def emit_pre(ib):
            # pre = gamma_c*softmax(..) @ cxq + cxq + sxq; the +cxq rides the
            # same PSUM via an identity-matmul accumulate
            psC = ps_m.tile([128, 512], fp, tag="m", name="m")
            cx = cxq[:, ib * 512:(ib + 1) * 512]
            nc.tensor.matmul(psC[:64, :], chan['ScT'], cx,
                             start=True, stop=False)
            nc.tensor.matmul(psC[:64, :], ident, cx, start=False, stop=True)
            nc.vector.tensor_tensor(pre[:, ib * 512:(ib + 1) * 512],
                                    psC[:64, :],
                                    sxq[:, ib * 512:(ib + 1) * 512], ALU.add)

        def emit_out(ib):
            fb = fuse[:, ib * 512:(ib + 1) * 512]
            nc.vector.tensor_add(fb, fb, pre[:, ib * 512:(ib + 1) * 512])
            psO = ps_m.tile([128, 512], fp, tag="m", name="m")
            nc.tensor.matmul(psO[:64, :], wo16, fb, start=True, stop=True)
            nc.vector.tensor_scalar_add(
                out_sb[:, ib * 512:(ib + 1) * 512], psO[:64, :], ob)
            nc.sync.dma_start(out_d[:, ib * 512:(ib + 1) * 512],
                              out_sb[:, ib * 512:(ib + 1) * 512])

        psS = {}

        def emit_S(r):
            ib, rnd = divmod(r, 16)
            ps = ps_s.tile([128, 1024], fp, tag="S", name="S")
            for hh in range(2):
                jo = rnd * 2 + hh
                nc.tensor.matmul(ps[:, hh * 512:(hh + 1) * 512],
                                 xd_dr[:, :, jo * 128:(jo + 1) * 128],
                                 qk_dr[:, :, ib * 512:(ib + 1) * 512],
                                 start=True, stop=True, perf_mode=DR)
            psS[r] = ps

        # ---------------- startup -------------------------------------------
        # PE p-state warmup: harmless matmuls on a zeroed scratch keep the
        # tensor engine busy while the first inputs stream in
        psW = ps_s.tile([128, 1024], fp, tag="S", name="S")
        for wi in range(3):
            nc.tensor.matmul(psW[:64, :512], wup[:64, :64], wup[:64, :512],
                             start=(wi == 0), stop=(wi == 2))
        emit_qk(0, 0)
        emit_qk(1, 0)
        emit_qk(0, 1)
        emit_qk(1, 1)
        emit_S(0)
        emit_S(1)
        emit_vt(0)

        fill_ps = {}

        def emit_fill(n):
            # dependency-free matmuls on the zeroed scratch keep PE's p-state
            # ramp alive across input-arrival bubbles
            if 'ps' not in fill_ps:
                fill_ps['ps'] = ps_s.tile([128, 1024], fp, tag="S", name="S")
                fill_ps['n'] = 0
            for _ in range(n):
                nc.tensor.matmul(fill_ps['ps'][:64, 512:1024],
                                 wup[:64, :64], wup[:64, :512],
                                 start=(fill_ps['n'] == 0), stop=False,
                                 skip_group_check=True)
                fill_ps['n'] += 1

        weave = {}

        def at(r, fn, *a):
            weave.setdefault(r, []).append((fn, a))

        at(0, emit_vt, 1)
        at(1, emit_vt, 2)
        at(2, emit_vt, 3)
        at(3, emit_vt, 4)
        for r in range(3):
            at(r, emit_fill, 2)
        at(3, emit_qconv, 0, wpk_s, sxq, sbias)
        at(4, emit_qconv, 1, wpk_s, sxq, sbias)
        at(5, emit_qconv, 2, wpk_s, sxq, sbias)
        at(6, emit_qconv, 0, wpk_c, cxq, cbias)
        at(7, emit_qconv, 1, wpk_c, cxq, cbias)
        at(8, emit_qconv, 2, wpk_c, cxq, cbias)
        for b in range(10):
            at(9 + b, emit_fblock, b)
        at(13, emit_ftrans, 0)
        at(15, emit_ftrans, 1)
        at(17, emit_ftrans, 2)
        at(19, emit_ftrans, 3)
        at(18, emit_gram, 0)
        at(19, emit_gram, 1)
        at(20, emit_gram, 2)
        at(21, emit_gram, 3)
        at(22, emit_chansoft)
        at(23, emit_pre, 0)
        at(23, emit_pre, 1)
        at(24, emit_out, 0)

        # ---------------- attention rounds ----------------------------------
        # U matmuls lag their round by 2 (Et is buffered 5 deep) so an early
        # U stall on vT/xd never blocks the S stream feeding ACT's exp pipe
        psU = {}
        Ets = {}

        def emit_U(rm):
            ibm, rndm = divmod(rm, 16)
            if rndm == 0:
                psU[ibm] = ps_u.tile([128, 512], fp, tag="U", name="U")
            nc.tensor.matmul(psU[ibm], vT[:, 2 * rndm:2 * rndm + 2, :],
                             Ets.pop(rm).rearrange("c (t n) -> c t n", t=2),
                             start=(rndm == 0), stop=(rndm == 15),
                             perf_mode=DR)

        def emit_tail0():
            rcp = small.tile([1, 512], f16, name="rec")
            with nc.allow_low_precision(reason="1/denom via f16 matmul"):
                nc.vector.reciprocal(rcp, psU[0][64:65, :])
            psB = ps_m.tile([128, 512], fp, tag="m", name="m")
            nc.tensor.matmul(psB[:64, :], ones16, rcp, start=True, stop=True)
            rec64 = small.tile([64, 512], f16, name="rec64")
            nc.vector.tensor_copy(rec64, psB[:64, :])
            fb = fuse[:, 0:512]
            nc.vector.tensor_tensor(fb, psU[0][:64, :], rec64, ALU.mult)

        for r in range(32):
            ib, rnd = divmod(r, 16)
            Et = work.tile([128, 1024], f8, tag="E", name="E")
            nc.scalar.activation(Et, psS.pop(r), ACTF.Exp)
            Ets[r] = Et
            if r >= 2:
                emit_U(r - 2)
                if r - 2 == 15:
                    emit_tail0()
            if r + 2 < 32:
                emit_S(r + 2)
            for fn, a in weave.get(r, ()):
                fn(*a)
        emit_U(30)
        emit_U(31)

        # ib1 tail, split into column halves; the reciprocal/broadcast-copy
        # links of one half run on ACT so the halves overlap across engines
        for h in range(2):
            cs = slice(512 + h * 256, 512 + (h + 1) * 256)
            us = slice(h * 256, (h + 1) * 256)
            rcp = small.tile([1, 256], f16, name="rec")
            with nc.allow_low_precision(reason="1/denom via f16 matmul"):
                nc.vector.reciprocal(rcp, psU[1][64:65, us])
            psB = ps_m.tile([128, 512], fp, tag="m", name="m")
            nc.tensor.matmul(psB[:64, :256], ones16, rcp,
                             start=True, stop=True)
            rec64 = small.tile([64, 256], f16, name="rec64")
            nc.vector.tensor_copy(rec64, psB[:64, :256])
            fb = fuse[:, cs]
            nc.vector.tensor_tensor(fb, psU[1][:64, us], rec64, ALU.mult)
            nc.vector.tensor_add(fb, fb, pre[:, cs])
            psO = ps_m.tile([128, 512], fp, tag="m", name="m")
            nc.tensor.matmul(psO[:64, :256], wo16, fb, start=True, stop=True)
            nc.vector.tensor_scalar_add(out_sb[:, cs], psO[:64, :256], ob)
            nc.sync.dma_start(out_d[:, cs], out_sb[:, cs])

    _postpass(nc)
    return nc


def _postpass(nc):
    """Enforce <=1 sync wait per engine instruction.

    Safe transforms only:
     - merge same-sem waits to the max target value;
     - drop a wait (sem, v) if an EARLIER same-engine instruction already
       guaranteed sem >= v (FIFO queues, monotone sems);
     - drop an own-engine wait >=4 updates back (pipeline drain ~190ns is
       far less than 4 instructions of engine occupancy);
     - drop a wait (sem, v) if another wait (sem2, v2) on the same
       instruction transitively covers it: the engine owning sem2 had
       already guaranteed sem >= v by the time its update count hit v2;
     - move a surplus wait to the immediately preceding same-engine
       instruction when the result still fits one wait slot per
       instruction: either merged into a same-sem wait there, or the host
       is an Ldweights / the moved wait is an input-DMA sem (input DMAs
       depend on nothing, Ldweights have no dependents - no cycles).
       Own-engine-sem waits never move (could self-deadlock).
    """
    import bass_rust
    eng_names = ('PE', 'Activation', 'DVE', 'Pool', 'SP')
    skip_types = ('InstEventSemaphore', 'InstDrain')
    sem_eng = {'PE_': 'PE', 'Activation_': 'Activation', 'DVE_': 'DVE',
               'Pool_': 'Pool'}

    def eng_of_sem(nm):
        for p, e in sem_eng.items():
            if nm.startswith(p):
                return e
        return None

    seen = {e: {} for e in eng_names}    # sem -> max value guaranteed
    snap = {e: {} for e in eng_names}    # update count -> seen snapshot
    cnt = {e: 0 for e in eng_names}      # cumulative own-sem update count
    prev = {e: None for e in eng_names}
    recent = {e: [] for e in eng_names}  # last few instrs (relocation hosts)
    last_dma_sem = None
    for blk in nc.m.functions[0].blocks:
        for ins in blk.instructions:
            tname = type(ins).__name__
            eng = str(getattr(ins, 'engine', '')).replace('EngineType.', '')
            si = ins.sync_info
            if si is not None:
                for u in si.on_update:
                    if u.ant_name.startswith('DMA'):
                        last_dma_sem = u.ant_name
            if eng not in eng_names or tname in skip_types:
                continue
            if eng == 'SP' and tname != 'InstDMACopy':
                continue
            if si is None:
                prev[eng] = ins
                recent[eng].append(ins)
                if len(recent[eng]) > 8:
                    recent[eng].pop(0)
                continue
            sn = seen[eng]
            merged = {}
            for w in si.on_wait:
                nm = w.ant_name
                v = w.wait_value or 0
                if nm not in merged or v > (merged[nm].wait_value or 0):
                    merged[nm] = w
            implied = dict(merged)
            kept = [w for w in merged.values()
                    if sn.get(w.ant_name, -1) < (w.wait_value or 0)]
            kept = [w for w in kept
                    if not (eng_of_sem(w.ant_name) == eng
                            and (w.wait_value or 0) <= cnt[eng] - 4)]
            if len(kept) > 1:
                def covered(w, others):
                    for o in others:
                        e2 = eng_of_sem(o.ant_name)
                        if e2 is None:
                            continue
                        s2 = snap[e2].get(o.wait_value or 0)
                        if s2 and s2.get(w.ant_name, -1) >= (w.wait_value or 0):
                            return True
                    return False
                kept = [w for w in kept
                        if not covered(w, [o for o in kept if o is not w])]
            for nm, w in implied.items():
                sn[nm] = max(sn.get(nm, -1), w.wait_value or 0)
            if len(kept) > 1:
                def host_for(move):
                    # immediate prev: same-sem merge or LW/free+DMA move
                    for depth, p in enumerate(reversed(recent[eng])):
                        p_w = list(p.sync_info.on_wait) if p.sync_info else []
                        p_names = {w.ant_name for w in p_w}
                        names = p_names | {w.ant_name for w in move}
                        ok_lw = type(p).__name__ == 'InstLdweights'
                        if depth == 0 and len(names) <= 1:
                            return p, p_w  # same-sem merge into prev
                        if not p_w and len(move) == 1 and (
                                (depth == 0 and ok_lw)
                                or move[0].ant_name.startswith('DMA')):
                            return p, p_w
                    return None, None
                plan = None
                for keep_w in kept:
                    move = [w for w in kept if w is not keep_w]
                    if any(eng_of_sem(w.ant_name) == eng for w in move):
                        continue  # never move own-engine waits
                    p, p_w = host_for(move)
                    if p is not None:
                        plan = (keep_w, move, p, p_w)
                        break
                assert plan is not None, \
                    (ins.name, eng, tname,
                     [(w.ant_name, w.wait_value) for w in kept])
                keep_w, move, p, p_w = plan
                newpw = {}
                for w in p_w + move:
                    nm = w.ant_name
                    if nm not in newpw or (w.wait_value or 0) > \
                            (newpw[nm].wait_value or 0):
                        newpw[nm] = w
                psi = p.sync_info
                if psi is None:
                    psi = bass_rust.SyncInfo(on_wait=[], on_update=[])
                psi.on_wait = list(newpw.values())
                p.sync_info = psi
                kept = [keep_w]
            si.on_wait = kept
            ins.sync_info = si
            prev[eng] = ins
            recent[eng].append(ins)
            if len(recent[eng]) > 8:
                recent[eng].pop(0)
            for u in si.on_update:
                if eng_of_sem(u.ant_name) == eng:
                    cnt[eng] += (u.update_value or 1)
                    snap[eng][cnt[eng]] = dict(seen[eng])
    # tail drains: the final out DMA transitively covers every engine
    for blk in nc.m.functions[0].blocks:
        for ins in blk.instructions:
            si = ins.sync_info
            if si is None or type(ins).__name__ != 'InstDrain':
                continue
            if len(si.on_wait) > 1 and last_dma_sem is not None:
                keep = [w for w in si.on_wait if w.ant_name == last_dma_sem]
                if keep:
                    si.on_wait = keep
                    ins.sync_info = si


def _prep_host(inputs):
    x = np.asarray(inputs['x'], np.float32)

    def fold(Wc, bc, g, b_, m, v):
        sc = np.asarray(g) / np.sqrt(np.asarray(v) + EPS)
        return (np.asarray(Wc) * sc[:, None, None, None],
                (np.asarray(bc) - np.asarray(m)) * sc + np.asarray(b_))

    sWf, sbf = fold(inputs['sW'], inputs['sb'], inputs['s_g'], inputs['s_b'],
                    inputs['s_m'], inputs['s_v'])
    cWf, cbf = fold(inputs['cW'], inputs['cb'], inputs['c_g'], inputs['c_b'],
                    inputs['c_m'], inputs['c_v'])

    def pack_pairs(Wf):
        # tap (dy,dx) weight matrix [cin, cout] = Wf[:, :, dy, dx].T
        T = [Wf[:, :, dy, dx].T for dy in range(3) for dx in range(3)]
        g = np.zeros((128, 5, 64), np.float32)
        g[:64, 0], g[64:, 0] = T[0], T[3]
        g[:64, 1], g[64:, 1] = T[1], T[4]
        g[:64, 2], g[64:, 2] = T[2], T[5]
        g[:64, 3], g[64:, 3] = T[6], T[7]
        g[:64, 4] = T[8]
        return g.reshape(128, 320)

    w16 = np.zeros((128, WLEN), np.float32)
    w16[:, O_WPS:O_WPS + 320] = pack_pairs(sWf)
    w16[:, O_WPC:O_WPC + 320] = pack_pairs(cWf)
    w16[:64, O_WO:O_WO + 64] = np.asarray(inputs['oW'])[:, :, 0, 0].T
    w16[:64, O_ID:O_ID + 64] = np.eye(64, dtype=np.float32)
    w16[0, O_ONE:O_ONE + 64] = 1.0
    w16 = w16.astype(np.float16)

    wk_full = np.zeros((65, Cq), np.float32)
    wk_full[:64] = np.asarray(inputs['kW'])[:, :, 0, 0].T
    wk_full[64] = np.asarray(inputs['kb'])
    wq_full = np.zeros((65, Cq), np.float32)
    wq_full[:64] = np.asarray(inputs['qW'])[:, :, 0, 0].T
    wq_full[64] = np.asarray(inputs['qb'])
    WKQ = np.zeros((66, 65), np.float32)
    WKQ[:65] = wk_full @ wq_full.T          # qk = WKQ @ xsd
    w0 = np.zeros((65, W0LEN), np.float32)
    for t in range(2):
        w0[:, O_KQ + t * 33:O_KQ + (t + 1) * 33] = WKQ[t * 33:(t + 1) * 33].T
    sg = float(np.asarray(inputs['s_gamma'])[0])
    w0[:64, O_WV:O_WV + 64] = np.asarray(inputs['vW'])[:, :, 0, 0].T * sg
    w0[64, O_WV:O_WV + 64] = np.asarray(inputs['vb']) * sg
    w0[64, O_WV + 64] = 1.0
    w0 = w0.astype(np.float16)

    c32 = np.zeros((64, 4), np.float32)
    c32[:, 0] = sbf
    c32[:, 1] = cbf
    c32[:, 2] = np.asarray(inputs['ob'])
    c32[:, 3] = float(np.asarray(inputs['c_gamma'])[0])

    in_maps = []
    for core in range(8):
        b, qi = core // 4, core % 4
        xb = x[b].astype(np.float16)
        xdm = np.ones((65, HW), np.float16)
        xdm[:64] = xb.reshape(64, HW)
        xsdm = np.ones((65, NQ), np.float16)
        xsdm[:64] = xb[:, qi * 16:(qi + 1) * 16, :].reshape(64, NQ)
        # padded flat with leading sentinel: z[1 + r*66 + w] = xpad[r, w]
        xp = np.zeros((64, 66, 66), np.float16)
        xp[:, 1:65, 1:65] = xb
        z = np.zeros((64, 4426 + 66), np.float16)
        z[:, 1:1 + 4356] = xp.reshape(64, 4356)
        sfm = np.zeros((128, LF), np.float16)
        sfm[:64] = z[:, 0:LF]
        sfm[64:] = z[:, 66:66 + LF]
        sf2m = np.zeros((128, LF2), np.float16)
        sf2m[:64] = z[:, 132:132 + LF2]
        sf2m[64:] = z[:, 133:133 + LF2]
        # our-quarter slab: padded rows [16qi, 16qi+18)
        zq = np.zeros((64, LQ + 200), np.float16)
        zq[:, 1:1 + 18 * 66] = xp[:, qi * 16:qi * 16 + 18, :].reshape(64, -1)
        sqm = np.zeros((128, LQ), np.float16)
        sqm[:64] = zq[:, 0:LQ]
        sqm[64:] = zq[:, 66:66 + LQ]
        sq2m = np.zeros((128, LQ2), np.float16)
        sq2m[:64] = zq[:, 132:132 + LQ2]
        sq2m[64:] = zq[:, 133:133 + LQ2]
        from ml_dtypes import float8_e4m3fn as _e4m3
        xpad66 = np.zeros((66, HW), np.float32)
        xpad66[:65] = xdm.astype(np.float32)
        xdr = np.zeros((33, 2, HW), np.float32)
        for t in range(2):
            xdr[:, t, :] = xpad66[t * 33:(t + 1) * 33]
        xdr = xdr.reshape(33, 2 * HW).astype(_e4m3)
        in_maps.append({
            'wpk0': w0, 'wpk': w16, 'cst': c32,
            'xd': np.ascontiguousarray(xdm),
            'xsd': np.ascontiguousarray(xsdm),
            'xdr': xdr,
            'sq': sqm, 'sq2': sq2m, 'sf': sfm, 'sf2': sf2m,
        })
    return in_maps


def kernel(**inputs):
    from concourse.bass_utils import run_bass_kernel_spmd
    if 'nc' not in _CACHE:
        _CACHE['nc'] = _build()
    nc = _CACHE['nc']
    in_maps = _prep_host(inputs)
    res = run_bass_kernel_spmd(nc, in_maps, core_ids=list(range(8)))
    out = np.zeros((B, C, H, W), np.float32)
    for c in range(8):
        b, qi = c // 4, c % 4
        out[b, :, qi * 16:(qi + 1) * 16, :] = \
            res.results[c]['out'].reshape(64, 16, 64)
    return out


# revision 7
# speedup vs baseline: 1.4054x; 1.0209x over previous
"""DSAM (dual spatial/channel attention) Bass kernel for Trainium2, 8 cores.

Sharding: core c handles batch b=c//4, query-row quarter qi=c%4
(1024 of the 4096 spatial positions). Spatial attention is fused
flash-style (scores -> exp -> weighted sum of V, normalization folded in
via an appended ones-row of V). The channel branch (full-image 3x3 conv +
64x64 gram) is computed redundantly per core.

Key optimizations over the 95.5us baseline:
 - host sends f16 data directly (no on-device dtype conversion);
 - doubled slabs [x | x<<66] and [x<<132 | x<<133] let every 3x3 conv run
   as 5 PE passes (4 tap-pairs with K=128 + 1 single) instead of 9, with
   conv bias folded into the DVE relu-eviction;
 - the whole spatial attention runs in fp8e4 with DoubleRow matmuls
   (2 contraction tiles per pass): scores contract k/q over [4,2,*]
   layouts, A@V contracts vT/exp(Et) over [128,2,*] - halving PE time;
   fp8 is numerically free here because the softmax self-normalizes;
 - input DMAs are HWDGE on the idle SP engine, ordered by first use;
   outputs stream per-half;
 - channel-branch work (conv blocks, transposes, gram, softmax) and the
   ca+cxq+sxq pre-sums are woven between attention rounds so PE/DVE run
   under the ACT exp stream (the long pole);
 - a post-pass enforces the 1-sync-wait-per-engine-instruction encoding
   limit (FIFO elision, transitive coverage, same-sem merge).

Hardcoded shapes: B=2, C=64, H=W=64, Cq=8.
"""

import numpy as np

EPS = 1e-5
B, C, H, W = 2, 64, 64, 64
HW = H * W
Cq = C // 8
NQ = 1024

LF = 4360
LF2 = 4224
LQ = 1192
LQ2 = 1058

# d_w0 column layout (f16 [65, W0LEN]) - tiny, arrives first
O_KQ = 0             # wkq_t [65, 2, 33]: (Wk @ Wq^T) halves, transposed
O_WV = 66            # [65, 65]
W0LEN = 131
# d_w column layout (f16 [128, WLEN])
O_WPS = 0            # wpk_s [128, 5, 64]
O_WPC = 320          # wpk_c [128, 5, 64]
O_WO = 640           # [64, 64]
O_ID = 704           # [64, 64]
O_ONE = 768          # [1, 64] ones
WLEN = 832

_CACHE = {}


def _build():
    import concourse.bass as bass
    import concourse.tile as tile
    from concourse import mybir
    from contextlib import ExitStack

    fp = mybir.dt.float32
    f16 = mybir.dt.float16
    f8 = mybir.dt.float8e4
    AX = mybir.AxisListType.X
    ALU = mybir.AluOpType
    ACTF = mybir.ActivationFunctionType
    DR = mybir.MatmulPerfMode.DoubleRow

    nc = bass.Bass()
    d_w0 = nc.dram_tensor("wpk0", [65, W0LEN], f16, kind="ExternalInput")
    d_w = nc.dram_tensor("wpk", [128, WLEN], f16, kind="ExternalInput")
    d_c = nc.dram_tensor("cst", [64, 4], fp, kind="ExternalInput")
    d_xd = nc.dram_tensor("xd", [65, HW], f16, kind="ExternalInput")
    d_xdr = nc.dram_tensor("xdr", [33, 2 * HW], mybir.dt.float8e4,
                           kind="ExternalInput")
    d_xsd = nc.dram_tensor("xsd", [65, NQ], f16, kind="ExternalInput")
    d_sq = nc.dram_tensor("sq", [128, LQ], f16, kind="ExternalInput")
    d_sq2 = nc.dram_tensor("sq2", [128, LQ2], f16, kind="ExternalInput")
    d_sf = nc.dram_tensor("sf", [128, LF], f16, kind="ExternalInput")
    d_sf2 = nc.dram_tensor("sf2", [128, LF2], f16, kind="ExternalInput")
    out_d = nc.dram_tensor("out", [64, NQ], fp, kind="ExternalOutput")

    with tile.TileContext(nc) as tc, ExitStack() as ctx:
        big = ctx.enter_context(tc.tile_pool(name="big", bufs=1))
        work = ctx.enter_context(tc.tile_pool(name="work", bufs=5))
        small = ctx.enter_context(tc.tile_pool(name="small", bufs=12))
        ps_s = ctx.enter_context(tc.tile_pool(name="ps_s", bufs=2, space="PSUM"))
        ps_u = ctx.enter_context(tc.tile_pool(name="ps_u", bufs=2, space="PSUM"))
        ps_m = ctx.enter_context(tc.tile_pool(name="ps_m", bufs=2, space="PSUM"))

        # ------------- input DMAs, desc-gen spread across idle queues -------
        # (HWDGE desc-gen costs ~650ns of SEQ time per DMA and serializes per
        # engine; the transfers themselves serialize on the DMA engines in
        # arrival order, so queue assignment = arrival priority)
        xsd = big.tile([65, NQ], f16)
        nc.sync.dma_start(xsd, d_xsd[:, :])
        w0_sb = big.tile([65, W0LEN], f16)
        nc.sync.dma_start(w0_sb, d_w0[:, :])
        xd_dr = big.tile([33, 2, HW], f8)
        nc.sync.dma_start(xd_dr,
                          d_xdr[:, :].rearrange("c (t n) -> c t n", t=2))
        c_sb = big.tile([64, 4], fp)
        nc.sync.dma_start(c_sb, d_c[:, :])
        xd = big.tile([65, HW], f16)
        nc.sync.dma_start(xd, d_xd[:, :])
        w_sb = big.tile([128, WLEN], f16)
        nc.sync.dma_start(w_sb, d_w[:, :])
        wup = big.tile([128, 512], f16)
        nc.gpsimd.memset(wup, 0.0)
        vT = big.tile([128, 32, 128], f8)
        nc.gpsimd.memset(vT[:, :, 65:128], 0.0)
        sq = big.tile([128, LQ], f16)
        nc.gpsimd.dma_start(sq, d_sq[:, :])
        sq2 = big.tile([128, LQ2], f16)
        nc.gpsimd.dma_start(sq2, d_sq2[:, :])
        sf = big.tile([128, LF], f16)
        nc.gpsimd.dma_start(sf, d_sf[:, :])
        sf2 = big.tile([128, LF2], f16)
        nc.gpsimd.dma_start(sf2, d_sf2[:, :])

        wpk_s = w_sb[:, O_WPS:O_WPS + 320].rearrange("c (g o) -> c g o", g=5)
        wpk_c = w_sb[:, O_WPC:O_WPC + 320].rearrange("c (g o) -> c g o", g=5)
        wv = w0_sb[:65, O_WV:O_WV + 65]
        wo16 = w_sb[:64, O_WO:O_WO + 64]
        ident = w_sb[:64, O_ID:O_ID + 64]
        ones16 = w_sb[0:1, O_ONE:O_ONE + 64]
        wkq = w0_sb[:65, O_KQ:O_KQ + 66].rearrange("c (t m) -> c t m", t=2)
        sbias = c_sb[:, 0:1]
        cbias = c_sb[:, 1:2]
        ob = c_sb[:, 2:3]
        cg = c_sb[:, 3:4]

        # early DVE touch: seeds FIFO wait-coverage of the d_c DMA so later
        # DVE instructions' d_c waits are elided by the post-pass
        cscr = big.tile([64, 4], fp)
        nc.vector.tensor_copy(cscr, c_sb)

        # persistent SBUF tensors
        qk_dr = big.tile([33, 2, NQ], f8)  # (Wk Wq^T xsd), DoubleRow layout
        cxf = big.tile([64, HW], f16)      # full-image channel fmap (dense)
        fT = big.tile([128, 32, 64], f16)  # fmap transposed chunks
        sxq = big.tile([64, NQ], f16)      # spatial-conv output, our rows
        cxq = big.tile([64, NQ], f16)      # channel-conv output, our rows
        pre = big.tile([64, NQ], f16)      # ca + cxq + sxq, precomputed
        fuse = big.tile([64, NQ], f16)
        out_sb = big.tile([64, NQ], fp)

        # ---------------- emitters ------------------------------------------
        def emit_qk(t, half):
            ps = ps_m.tile([128, 512], fp, tag="m", name="m")
            nc.tensor.matmul(ps[:33, :], wkq[:, t, :33],
                             xsd[:, half * 512:(half + 1) * 512],
                             start=True, stop=True)
            nc.vector.tensor_copy(qk_dr[:, t, half * 512:(half + 1) * 512],
                                  ps[:33, :])

        def emit_vt(grp):
            n_t = min(7, 32 - grp * 7)
            ps = ps_m.tile([128, 512], fp, tag="m", name="m")
            for t in range(n_t):
                jo = grp * 7 + t
                nc.tensor.matmul(ps[:, t * 65:(t + 1) * 65],
                                 xd[:, jo * 128:(jo + 1) * 128], wv,
                                 start=True, stop=True)
            nc.vector.tensor_copy(vT[:, grp * 7:grp * 7 + n_t, 0:65],
                                  ps[:, :n_t * 65])

        # paired-tap 3x3 conv block: 5 matmuls. sA = [x | x<<66] slab,
        # sB = [x<<132 | x<<133] slab. The sB pair is emitted last so a
        # late-arriving sB DMA doesn't stall the earlier taps.
        def conv_mms(ps, wpk, sA, sB, base, fsz):
            nc.tensor.matmul(ps[:64, :fsz], wpk[:, 0, :],
                             sA[:, base: base + fsz], start=True, stop=False)
            nc.tensor.matmul(ps[:64, :fsz], wpk[:, 1, :],
                             sA[:, base + 1: base + 1 + fsz],
                             start=False, stop=False)
            nc.tensor.matmul(ps[:64, :fsz], wpk[:, 2, :],
                             sA[:, base + 2: base + 2 + fsz],
                             start=False, stop=False)
            nc.tensor.matmul(ps[:64, :fsz], wpk[:64, 4, :],
                             sA[:64, base + 134: base + 134 + fsz],
                             start=False, stop=False)
            nc.tensor.matmul(ps[:64, :fsz], wpk[:, 3, :],
                             sB[:, base: base + fsz], start=False, stop=True)

        F_ROWS = [7] * 9 + [1]

        def emit_fblock(b):
            rows = F_ROWS[b]
            done = 7 * b
            fsz = rows * 66
            ps = ps_m.tile([128, 512], fp, tag="m", name="m")
            conv_mms(ps, wpk_c, sf, sf2, done * 66, fsz)
            pv = ps[:64, :fsz].rearrange("c (r w) -> c r w", w=66)[:, :, 1:65]
            nc.vector.tensor_scalar(
                cxf[:, done * 64:(done + rows) * 64].rearrange(
                    "c (r w) -> c r w", w=64),
                pv, cbias, 0.0, ALU.add, ALU.max)

        def emit_qconv(bi, wpk, dst, bias):
            rows = (7, 7, 2)[bi]
            p0 = (0, 462, 924)[bi]
            fsz = rows * 66
            ps = ps_m.tile([128, 512], fp, tag="m", name="m")
            conv_mms(ps, wpk, sq, sq2, p0, fsz)
            pv = ps[:64, :fsz].rearrange("c (r w) -> c r w", w=66)[:, :, 1:65]
            nc.vector.tensor_scalar(
                dst[:, p0 // 66 * 64:(p0 // 66 + rows) * 64].rearrange(
                    "c (r w) -> c r w", w=64),
                pv, bias, 0.0, ALU.add, ALU.max)

        def emit_ftrans(grp):
            ps = ps_m.tile([128, 512], f16, tag="m", name="m")
            for t in range(8):
                jo = grp * 8 + t
                nc.tensor.transpose(ps[:, t * 64:(t + 1) * 64],
                                    cxf[:, jo * 128:(jo + 1) * 128], ident)
            nc.vector.tensor_copy(fT[:, grp * 8:(grp + 1) * 8, :], ps)

        gram_ps = {}

        def emit_gram(grp):
            if 'ps' not in gram_ps:
                gram_ps['ps'] = ps_u.tile([128, 512], fp, tag="U", name="U")
            psA = gram_ps['ps'][:64, :64]
            for t in range(8):
                jo = grp * 8 + t
                nc.tensor.matmul(psA, fT[:, jo, :], fT[:, jo, :],
                                 start=(jo == 0), stop=(jo == 31))

        chan = {}

        def emit_chansoft():
            psA = gram_ps['ps'][:64, :64]
            Ac = small.tile([64, 64], fp, name="ac")
            nc.vector.tensor_copy(Ac, psA)
            mn = small.tile([64, 1], fp, name="mn")
            nc.vector.tensor_reduce(mn, Ac, AX, ALU.min)
            Ec = small.tile([64, 64], f16, name="ec")
            # exp(mn - Ac): softmax(max-Ac) == softmax(-Ac), stabilized by min
            nc.scalar.activation(Ec, Ac, ACTF.Exp, bias=mn, scale=-1.0)
            sm = small.tile([64, 1], fp, name="sm")
            nc.vector.reduce_sum(sm, Ec, AX)
            rc = small.tile([64, 1], fp, name="rc")
            nc.vector.reciprocal(rc, sm)
            # Ec := Ec * (1/sum) * c_gamma
            nc.vector.tensor_scalar(Ec, Ec, rc, cg, ALU.mult, ALU.mult)
            psT = ps_m.tile([128, 512], f16, tag="m", name="m")
            nc.tensor.transpose(psT[:64, :64], Ec, ident)
            ScT = small.tile([64, 64], f16, name="sct")
            nc.vector.tensor_copy(ScT, psT[:64, :64])
            chan['ScT'] = ScT

        def emit_pre(ib):
            # pre = gamma_c*softmax(..) @ cxq + cxq + sxq; the +cxq rides the
            # same PSUM via an identity-matmul accumulate
            psC = ps_m.tile([128, 512], fp, tag="m", name="m")
            cx = cxq[:, ib * 512:(ib + 1) * 512]
            nc.tensor.matmul(psC[:64, :], chan['ScT'], cx,
                             start=True, stop=False)
            nc.tensor.matmul(psC[:64, :], ident, cx, start=False, stop=True)
            nc.vector.tensor_tensor(pre[:, ib * 512:(ib + 1) * 512],
                                    psC[:64, :],
                                    sxq[:, ib * 512:(ib + 1) * 512], ALU.add)

        def emit_out(ib):
            fb = fuse[:, ib * 512:(ib + 1) * 512]
            nc.vector.tensor_add(fb, fb, pre[:, ib * 512:(ib + 1) * 512])
            psO = ps_m.tile([128, 512], fp, tag="m", name="m")
            nc.tensor.matmul(psO[:64, :], wo16, fb, start=True, stop=True)
            nc.vector.tensor_scalar_add(
                out_sb[:, ib * 512:(ib + 1) * 512], psO[:64, :], ob)
            nc.sync.dma_start(out_d[:, ib * 512:(ib + 1) * 512],
                              out_sb[:, ib * 512:(ib + 1) * 512])

        psS = {}

        def emit_S(r):
            ib, rnd = divmod(r, 16)
            ps = ps_s.tile([128, 1024], fp, tag="S", name="S")
            for hh in range(2):
                jo = rnd * 2 + hh
                nc.tensor.matmul(ps[:, hh * 512:(hh + 1) * 512],
                                 xd_dr[:, :, jo * 128:(jo + 1) * 128],
                                 qk_dr[:, :, ib * 512:(ib + 1) * 512],
                                 start=True, stop=True, perf_mode=DR)
            psS[r] = ps

        # ---------------- startup -------------------------------------------
        # PE p-state warmup: harmless matmuls on a zeroed scratch keep the
        # tensor engine busy while the first inputs stream in
        psW = ps_s.tile([128, 1024], fp, tag="S", name="S")
        for wi in range(3):
            nc.tensor.matmul(psW[:64, :512], wup[:64, :64], wup[:64, :512],
                             start=(wi == 0), stop=(wi == 2))
        emit_qk(0, 0)
        emit_qk(1, 0)
        emit_S(0)
        emit_S(1)
        emit_qk(0, 1)
        emit_qk(1, 1)

        fill_ps = {}

        def emit_fill(n):
            # dependency-free matmuls on the zeroed scratch keep PE's p-state
            # ramp alive across input-arrival bubbles
            if 'ps' not in fill_ps:
                fill_ps['ps'] = ps_s.tile([128, 1024], fp, tag="S", name="S")
                fill_ps['n'] = 0
            for _ in range(n):
                nc.tensor.matmul(fill_ps['ps'][:64, 512:1024],
                                 wup[:64, :64], wup[:64, :512],
                                 start=(fill_ps['n'] == 0), stop=False,
                                 skip_group_check=True)
                fill_ps['n'] += 1

        weave = {}

        def at(r, fn, *a):
            weave.setdefault(r, []).append((fn, a))

        at(0, emit_vt, 0)
        at(0, emit_vt, 1)
        at(1, emit_vt, 2)
        at(2, emit_vt, 3)
        at(3, emit_vt, 4)
        for r in range(3):
            at(r, emit_fill, 2)
        at(3, emit_qconv, 0, wpk_s, sxq, sbias)
        at(4, emit_qconv, 1, wpk_s, sxq, sbias)
        at(5, emit_qconv, 2, wpk_s, sxq, sbias)
        at(6, emit_qconv, 0, wpk_c, cxq, cbias)
        at(7, emit_qconv, 1, wpk_c, cxq, cbias)
        at(8, emit_qconv, 2, wpk_c, cxq, cbias)
        for b in range(10):
            at(9 + b, emit_fblock, b)
        at(13, emit_ftrans, 0)
        at(15, emit_ftrans, 1)
        at(17, emit_ftrans, 2)
        at(19, emit_ftrans, 3)
        at(18, emit_gram, 0)
        at(19, emit_gram, 1)
        at(20, emit_gram, 2)
        at(21, emit_gram, 3)
        at(22, emit_chansoft)
        at(23, emit_pre, 0)
        at(23, emit_pre, 1)
        at(24, emit_out, 0)

        # ---------------- attention rounds ----------------------------------
        # U matmuls lag their round by 2 (Et is buffered 5 deep) so an early
        # U stall on vT/xd never blocks the S stream feeding ACT's exp pipe
        psU = {}
        Ets = {}

        def emit_U(rm):
            ibm, rndm = divmod(rm, 16)
            if rndm == 0:
                psU[ibm] = ps_u.tile([128, 512], fp, tag="U", name="U")
            nc.tensor.matmul(psU[ibm], vT[:, 2 * rndm:2 * rndm + 2, :],
                             Ets.pop(rm).rearrange("c (t n) -> c t n", t=2),
                             start=(rndm == 0), stop=(rndm == 15),
                             perf_mode=DR)

        def emit_tail0():
            rcp = small.tile([1, 512], f16, name="rec")
            with nc.allow_low_precision(reason="1/denom via f16 matmul"):
                nc.vector.reciprocal(rcp, psU[0][64:65, :])
            psB = ps_m.tile([128, 512], fp, tag="m", name="m")
            nc.tensor.matmul(psB[:64, :], ones16, rcp, start=True, stop=True)
            rec64 = small.tile([64, 512], f16, name="rec64")
            nc.vector.tensor_copy(rec64, psB[:64, :])
            fb = fuse[:, 0:512]
            nc.vector.tensor_tensor(fb, psU[0][:64, :], rec64, ALU.mult)

        for r in range(32):
            ib, rnd = divmod(r, 16)
            Et = work.tile([128, 1024], f8, tag="E", name="E")
            nc.scalar.activation(Et, psS.pop(r), ACTF.Exp)
            Ets[r] = Et
            if r >= 2:
                emit_U(r - 2)
                if r - 2 == 15:
                    emit_tail0()
            if r + 2 < 32:
                emit_S(r + 2)
            for fn, a in weave.get(r, ()):
                fn(*a)
        emit_U(30)
        emit_U(31)

        # ib1 tail, split into column halves; the reciprocal/broadcast-copy
        # links of one half run on ACT so the halves overlap across engines
        for h in range(2):
            cs = slice(512 + h * 256, 512 + (h + 1) * 256)
            us = slice(h * 256, (h + 1) * 256)
            rcp = small.tile([1, 256], f16, name="rec")
            with nc.allow_low_precision(reason="1/denom via f16 matmul"):
                nc.vector.reciprocal(rcp, psU[1][64:65, us])
            psB = ps_m.tile([128, 512], fp, tag="m", name="m")
            nc.tensor.matmul(psB[:64, :256], ones16, rcp,
                             start=True, stop=True)
            rec64 = small.tile([64, 256], f16, name="rec64")
            nc.vector.tensor_copy(rec64, psB[:64, :256])
            fb = fuse[:, cs]
            nc.vector.tensor_tensor(fb, psU[1][:64, us], rec64, ALU.mult)
            nc.vector.tensor_add(fb, fb, pre[:, cs])
            psO = ps_m.tile([128, 512], fp, tag="m", name="m")
            nc.tensor.matmul(psO[:64, :256], wo16, fb, start=True, stop=True)
            nc.vector.tensor_scalar_add(out_sb[:, cs], psO[:64, :256], ob)
            nc.sync.dma_start(out_d[:, cs], out_sb[:, cs])

    _postpass(nc)
    return nc


def _postpass(nc):
    """Enforce <=1 sync wait per engine instruction.

    Safe transforms only:
     - merge same-sem waits to the max target value;
     - drop a wait (sem, v) if an EARLIER same-engine instruction already
       guaranteed sem >= v (FIFO queues, monotone sems);
     - drop an own-engine wait >=4 updates back (pipeline drain ~190ns is
       far less than 4 instructions of engine occupancy);
     - drop a wait (sem, v) if another wait (sem2, v2) on the same
       instruction transitively covers it: the engine owning sem2 had
       already guaranteed sem >= v by the time its update count hit v2;
     - move a surplus wait to the immediately preceding same-engine
       instruction when the result still fits one wait slot per
       instruction: either merged into a same-sem wait there, or the host
       is an Ldweights / the moved wait is an input-DMA sem (input DMAs
       depend on nothing, Ldweights have no dependents - no cycles).
       Own-engine-sem waits never move (could self-deadlock).
    """
    import bass_rust
    eng_names = ('PE', 'Activation', 'DVE', 'Pool', 'SP')
    skip_types = ('InstEventSemaphore', 'InstDrain')
    sem_eng = {'PE_': 'PE', 'Activation_': 'Activation', 'DVE_': 'DVE',
               'Pool_': 'Pool'}

    def eng_of_sem(nm):
        for p, e in sem_eng.items():
            if nm.startswith(p):
                return e
        return None

    seen = {e: {} for e in eng_names}    # sem -> max value guaranteed
    snap = {e: {} for e in eng_names}    # update count -> seen snapshot
    cnt = {e: 0 for e in eng_names}      # cumulative own-sem update count
    prev = {e: None for e in eng_names}
    recent = {e: [] for e in eng_names}  # last few instrs (relocation hosts)
    last_dma_sem = None
    for blk in nc.m.functions[0].blocks:
        for ins in blk.instructions:
            tname = type(ins).__name__
            eng = str(getattr(ins, 'engine', '')).replace('EngineType.', '')
            si = ins.sync_info
            if si is not None:
                for u in si.on_update:
                    if u.ant_name.startswith('DMA'):
                        last_dma_sem = u.ant_name
            if eng not in eng_names or tname in skip_types:
                continue
            if eng == 'SP' and tname != 'InstDMACopy':
                continue
            if si is None:
                prev[eng] = ins
                recent[eng].append(ins)
                if len(recent[eng]) > 8:
                    recent[eng].pop(0)
                continue
            sn = seen[eng]
            merged = {}
            for w in si.on_wait:
                nm = w.ant_name
                v = w.wait_value or 0
                if nm not in merged or v > (merged[nm].wait_value or 0):
                    merged[nm] = w
            implied = dict(merged)
            kept = [w for w in merged.values()
                    if sn.get(w.ant_name, -1) < (w.wait_value or 0)]
            kept = [w for w in kept
                    if not (eng_of_sem(w.ant_name) == eng
                            and (w.wait_value or 0) <= cnt[eng] - 4)]
            if len(kept) > 1:
                def covered(w, others):
                    for o in others:
                        e2 = eng_of_sem(o.ant_name)
                        if e2 is None:
                            continue
                        s2 = snap[e2].get(o.wait_value or 0)
                        if s2 and s2.get(w.ant_name, -1) >= (w.wait_value or 0):
                            return True
                    return False
                kept = [w for w in kept
                        if not covered(w, [o for o in kept if o is not w])]
            for nm, w in implied.items():
                sn[nm] = max(sn.get(nm, -1), w.wait_value or 0)
            if len(kept) > 1:
                def host_for(move):
                    # immediate prev: same-sem merge or LW/free+DMA move
                    for depth, p in enumerate(reversed(recent[eng])):
                        p_w = list(p.sync_info.on_wait) if p.sync_info else []
                        p_names = {w.ant_name for w in p_w}
                        names = p_names | {w.ant_name for w in move}
                        ok_lw = type(p).__name__ == 'InstLdweights'
                        if depth == 0 and len(names) <= 1:
                            return p, p_w  # same-sem merge into prev
                        if not p_w and len(move) == 1 and (
                                (depth == 0 and ok_lw)
                                or move[0].ant_name.startswith('DMA')):
                            return p, p_w
                    return None, None
                plan = None
                for keep_w in kept:
                    move = [w for w in kept if w is not keep_w]
                    if any(eng_of_sem(w.ant_name) == eng for w in move):
                        continue  # never move own-engine waits
                    p, p_w = host_for(move)
                    if p is not None:
                        plan = (keep_w, move, p, p_w)
                        break
                assert plan is not None, \
                    (ins.name, eng, tname,
                     [(w.ant_name, w.wait_value) for w in kept])
                keep_w, move, p, p_w = plan
                newpw = {}
                for w in p_w + move:
                    nm = w.ant_name
                    if nm not in newpw or (w.wait_value or 0) > \
                            (newpw[nm].wait_value or 0):
                        newpw[nm] = w
                psi = p.sync_info
                if psi is None:
                    psi = bass_rust.SyncInfo(on_wait=[], on_update=[])
                psi.on_wait = list(newpw.values())
                p.sync_info = psi
                kept = [keep_w]
            si.on_wait = kept
            ins.sync_info = si
            prev[eng] = ins
            recent[eng].append(ins)
            if len(recent[eng]) > 8:
                recent[eng].pop(0)
            for u in si.on_update:
                if eng_of_sem(u.ant_name) == eng:
                    cnt[eng] += (u.update_value or 1)
                    snap[eng][cnt[eng]] = dict(seen[eng])
    # tail drains: the final out DMA transitively covers every engine
    for blk in nc.m.functions[0].blocks:
        for ins in blk.instructions:
            si = ins.sync_info
            if si is None or type(ins).__name__ != 'InstDrain':
                continue
            if len(si.on_wait) > 1 and last_dma_sem is not None:
                keep = [w for w in si.on_wait if w.ant_name == last_dma_sem]
                if keep:
                    si.on_wait = keep
                    ins.sync_info = si


def _prep_host(inputs):
    x = np.asarray(inputs['x'], np.float32)

    def fold(Wc, bc, g, b_, m, v):
        sc = np.asarray(g) / np.sqrt(np.asarray(v) + EPS)
        return (np.asarray(Wc) * sc[:, None, None, None],
                (np.asarray(bc) - np.asarray(m)) * sc + np.asarray(b_))

    sWf, sbf = fold(inputs['sW'], inputs['sb'], inputs['s_g'], inputs['s_b'],
                    inputs['s_m'], inputs['s_v'])
    cWf, cbf = fold(inputs['cW'], inputs['cb'], inputs['c_g'], inputs['c_b'],
                    inputs['c_m'], inputs['c_v'])

    def pack_pairs(Wf):
        # tap (dy,dx) weight matrix [cin, cout] = Wf[:, :, dy, dx].T
        T = [Wf[:, :, dy, dx].T for dy in range(3) for dx in range(3)]
        g = np.zeros((128, 5, 64), np.float32)
        g[:64, 0], g[64:, 0] = T[0], T[3]
        g[:64, 1], g[64:, 1] = T[1], T[4]
        g[:64, 2], g[64:, 2] = T[2], T[5]
        g[:64, 3], g[64:, 3] = T[6], T[7]
        g[:64, 4] = T[8]
        return g.reshape(128, 320)

    w16 = np.zeros((128, WLEN), np.float32)
    w16[:, O_WPS:O_WPS + 320] = pack_pairs(sWf)
    w16[:, O_WPC:O_WPC + 320] = pack_pairs(cWf)
    w16[:64, O_WO:O_WO + 64] = np.asarray(inputs['oW'])[:, :, 0, 0].T
    w16[:64, O_ID:O_ID + 64] = np.eye(64, dtype=np.float32)
    w16[0, O_ONE:O_ONE + 64] = 1.0
    w16 = w16.astype(np.float16)

    wk_full = np.zeros((65, Cq), np.float32)
    wk_full[:64] = np.asarray(inputs['kW'])[:, :, 0, 0].T
    wk_full[64] = np.asarray(inputs['kb'])
    wq_full = np.zeros((65, Cq), np.float32)
    wq_full[:64] = np.asarray(inputs['qW'])[:, :, 0, 0].T
    wq_full[64] = np.asarray(inputs['qb'])
    WKQ = np.zeros((66, 65), np.float32)
    WKQ[:65] = wk_full @ wq_full.T          # qk = WKQ @ xsd
    w0 = np.zeros((65, W0LEN), np.float32)
    for t in range(2):
        w0[:, O_KQ + t * 33:O_KQ + (t + 1) * 33] = WKQ[t * 33:(t + 1) * 33].T
    sg = float(np.asarray(inputs['s_gamma'])[0])
    w0[:64, O_WV:O_WV + 64] = np.asarray(inputs['vW'])[:, :, 0, 0].T * sg
    w0[64, O_WV:O_WV + 64] = np.asarray(inputs['vb']) * sg
    w0[64, O_WV + 64] = 1.0
    w0 = w0.astype(np.float16)

    c32 = np.zeros((64, 4), np.float32)
    c32[:, 0] = sbf
    c32[:, 1] = cbf
    c32[:, 2] = np.asarray(inputs['ob'])
    c32[:, 3] = float(np.asarray(inputs['c_gamma'])[0])

    in_maps = []
    for core in range(8):
        b, qi = core // 4, core % 4
        xb = x[b].astype(np.float16)
        xdm = np.ones((65, HW), np.float16)
        xdm[:64] = xb.reshape(64, HW)
        xsdm = np.ones((65, NQ), np.float16)
        xsdm[:64] = xb[:, qi * 16:(qi + 1) * 16, :].reshape(64, NQ)
        # padded flat with leading sentinel: z[1 + r*66 + w] = xpad[r, w]
        xp = np.zeros((64, 66, 66), np.float16)
        xp[:, 1:65, 1:65] = xb
        z = np.zeros((64, 4426 + 66), np.float16)
        z[:, 1:1 + 4356] = xp.reshape(64, 4356)
        sfm = np.zeros((128, LF), np.float16)
        sfm[:64] = z[:, 0:LF]
        sfm[64:] = z[:, 66:66 + LF]
        sf2m = np.zeros((128, LF2), np.float16)
        sf2m[:64] = z[:, 132:132 + LF2]
        sf2m[64:] = z[:, 133:133 + LF2]
        # our-quarter slab: padded rows [16qi, 16qi+18)
        zq = np.zeros((64, LQ + 200), np.float16)
        zq[:, 1:1 + 18 * 66] = xp[:, qi * 16:qi * 16 + 18, :].reshape(64, -1)
        sqm = np.zeros((128, LQ), np.float16)
        sqm[:64] = zq[:, 0:LQ]
        sqm[64:] = zq[:, 66:66 + LQ]
        sq2m = np.zeros((128, LQ2), np.float16)
        sq2m[:64] = zq[:, 132:132 + LQ2]
        sq2m[64:] = zq[:, 133:133 + LQ2]
        from ml_dtypes import float8_e4m3fn as _e4m3
        xpad66 = np.zeros((66, HW), np.float32)
        xpad66[:65] = xdm.astype(np.float32)
        xdr = np.zeros((33, 2, HW), np.float32)
        for t in range(2):
            xdr[:, t, :] = xpad66[t * 33:(t + 1) * 33]
        xdr = xdr.reshape(33, 2 * HW).astype(_e4m3)
        in_maps.append({
            'wpk0': w0, 'wpk': w16, 'cst': c32,
            'xd': np.ascontiguousarray(xdm),
            'xsd': np.ascontiguousarray(xsdm),
            'xdr': xdr,
            'sq': sqm, 'sq2': sq2m, 'sf': sfm, 'sf2': sf2m,
        })
    return in_maps


def kernel(**inputs):
    from concourse.bass_utils import run_bass_kernel_spmd
    if 'nc' not in _CACHE:
        _CACHE['nc'] = _build()
    nc = _CACHE['nc']
    in_maps = _prep_host(inputs)
    res = run_bass_kernel_spmd(nc, in_maps, core_ids=list(range(8)))
    out = np.zeros((B, C, H, W), np.float32)
    for c in range(8):
        b, qi = c // 4, c % 4
        out[b, :, qi * 16:(qi + 1) * 16, :] = \
            res.results[c]['out'].reshape(64, 16, 64)
    return out
